# revision 9
# baseline (speedup 1.0000x reference)
"""Trainium2 Bass kernel for nn_MoEModel_18476949307967.

4-layer MoE transformer: B=2, S=2048, D=1024, E=8 experts top-2, H=8 heads,
3 interleaved attention layers. 8-way data-parallel over tokens (4096 tokens
-> 512/core; cores 0-3 = batch 0, cores 4-7 = batch 1). Attention K/V is
all-gathered within each 4-core batch group. Dense all-expert FFN per core
with top-2 masked combine (matches the reference exactly). Matmuls run in
float32r (tf32-like, full PE rate); the residual stream stays fp32.

Self-contained: hardcodes all shapes; no imports from the problem dir.
"""
from contextlib import ExitStack

import numpy as np

import concourse.bass as bass
import concourse.mybir as mybir
import concourse.tile as tile
from concourse import bacc, bass_utils
from concourse.bass import ts, ds

F32 = mybir.dt.float32
# Matmul operand dtype: float32 is exact (matches the reference's expert
# top-2 selections bit-for-bit in practice) at 4 PE cycles/row; float32r
# (tf32-like) runs 4x faster but its ~2e-4 logit error flips a couple of
# near-tied top-2 router selections vs the fp32 reference.
F32R = mybir.dt.float32
AF = mybir.ActivationFunctionType
ALU = mybir.AluOpType
AX = mybir.AxisListType

B, S, D, E, K, L, H = 2, 2048, 1024, 8, 2, 4, 8
HD = D // H                       # 128
NC = 8                            # cores
TC = (B * S) // NC                # 512 tokens per core
TT = TC // 128                    # 4 token tiles
KC = D // 128                     # 8 contraction subtiles
GROUP = [[0, 1, 2, 3], [4, 5, 6, 7]]
INV_SQRT_HD = 1.0 / float(np.sqrt(HD))
EPS = 1e-5
KV_HALF = D * TC

_CACHE = {}


class _Ctx:
    pass


def _load_wh(c, w2d, col_off, kh):
    """[128, 4, 512] f32r tile <- w2d[kh*512:(kh+1)*512, col_off:col_off+512]."""
    t = c.wexp.tile([128, 4, 512], F32R, tag="wh")
    c.nc.sync.dma_start(
        t, w2d[ds(kh * 512, 512), ds(col_off, 512)]
        .rearrange("(kc p) f -> p kc f", p=128).bitcast(F32R))
    return t


def _mm8(nc, ps, tA, tB, lhs_fn, rhs_fn, tail=None):
    """ps = sum_{kc<8} lhs(t, kc).T @ rhs(kc), lhsT slices from tA/tB halves."""
    for kc in range(KC):
        t = tA if kc < 4 else tB
        last = (kc == KC - 1) and tail is None
        nc.tensor.matmul(ps, lhsT=lhs_fn(t, kc), rhs=rhs_fn(t, kc),
                         start=(kc == 0), stop=last)
    if tail is not None:
        nc.tensor.matmul(ps, lhsT=tail[0], rhs=tail[1], start=False, stop=True)


def _transpose_nat_to_T(c, src_nat, dst_T):
    nc = c.nc
    for m in range(KC):
        for tt in range(TT):
            pt = c.psA.tile([128, 512], F32, tag="psA")
            nc.tensor.transpose(pt[:, :128], src_nat[:, tt, ts(m, 128)], c.eye)
            nc.vector.tensor_copy(dst_T[:, m, ts(tt, 128)], pt[:, :128])


def _input_proj(c):
    nc = c.nc
    x0 = c.xtp.tile([128, KC, TC], F32R, tag="bigT")
    nc.sync.dma_start(
        x0, c.xt0_d.rearrange("(kc p) t -> p kc t", p=128).bitcast(F32R))
    xT = c.xtp.tile([128, KC, TC], F32R, tag="bigT")
    for half in range(2):
        tA = _load_wh(c, c.wi_d, half * 512, 0)
        tB = _load_wh(c, c.wi_d, half * 512, 1)
        for mm in range(4):
            m = half * 4 + mm
            ps = c.psA.tile([128, 512], F32, tag="psA")
            _mm8(nc, ps, tA, tB,
                 lambda t, k, _m=mm: t[:, k % 4, ts(_m, 128)],
                 lambda t, k: x0[:, k, :])
            nc.vector.tensor_scalar(
                xT[:, m, :], ps, c.ibt[:, m:m + 1], None, op0=ALU.add)
    x_n = c.xnat.tile([128, TT, D], F32, tag="xnat")
    for m in range(KC):
        for tt in range(TT):
            pt = c.psA.tile([128, 512], F32, tag="psA")
            nc.tensor.matmul(pt[:, :128], lhsT=xT[:, m, ts(tt, 128)],
                             rhs=c.eyer, start=True, stop=True)
            nc.vector.tensor_copy(x_n[:, tt, ts(m, 128)], pt[:, :128])
    return xT, x_n


def _router(c, l, xT):
    nc = c.nc
    rw = c.small.tile([128, KC, E], F32R, tag="rw")
    nc.sync.dma_start(
        rw, c.rw_d[l].rearrange("(kc p) e -> p kc e", p=128).bitcast(F32R))
    rbb = c.small.tile([128, E], F32, tag="rbb")
    nc.sync.dma_start(rbb, c.rbb_d[l])
    coef = c.small.tile([128, TT, E], F32, tag="coef")
    for tt in range(TT):
        psl = c.psA.tile([128, 512], F32, tag="psA")
        for k in range(KC):
            nc.tensor.matmul(psl[:, :E], lhsT=xT[:, k, ts(tt, 128)],
                             rhs=rw[:, k, :], start=(k == 0),
                             stop=(k == KC - 1))
        lgt = c.small.tile([128, E], F32, tag="lgt")
        nc.vector.tensor_tensor(lgt, psl[:, :E], rbb, op=ALU.add)
        nc.sync.dma_start(c.lg_out[l, ts(tt, 128), :], lgt)
        m8 = c.small.tile([128, 8], F32, tag="m8")
        nc.vector.max(m8, lgt)
        negmax = c.small.tile([128, 1], F32, tag="negmax")
        nc.vector.tensor_scalar_mul(negmax, m8[:, 0:1], -1.0)
        elg = c.small.tile([128, E], F32, tag="elg")
        denl = c.small.tile([128, 1], F32, tag="denl")
        nc.scalar.activation(elg, lgt, AF.Exp, bias=negmax, scale=1.0,
                             accum_out=denl)
        rcl = c.small.tile([128, 1], F32, tag="rcl")
        nc.vector.reciprocal(rcl, denl)
        probs = c.small.tile([128, E], F32, tag="probs")
        nc.vector.tensor_scalar_mul(probs, elg, rcl)
        nc.vector.scalar_tensor_tensor(
            coef[:, tt, :], lgt, m8[:, 1:2], probs,
            op0=ALU.is_ge, op1=ALU.mult)
    return coef


def _moe_experts(c, l, xT, coef):
    nc = c.nc
    y = c.natp.tile([128, TT, D], F32, tag="nat")
    for e in range(E):
        b1t = c.small.tile([128, KC], F32, tag="b1t")
        nc.sync.dma_start(b1t, c.b1_d[l][e])
        b2r = c.rows.tile([1, D], F32R, tag="b2r")
        nc.sync.dma_start(b2r, c.b2_d[l][e, None, :].bitcast(F32R))
        hT = c.htp.tile([128, KC, TC], F32R, tag="hT")
        w1 = c.w1_d[l][e]
        for half in range(2):
            tA = _load_wh(c, w1, half * 512, 0)
            tB = _load_wh(c, w1, half * 512, 1)
            for mm in range(4):
                m = half * 4 + mm
                ps = c.psA.tile([128, 512], F32, tag="psA")
                _mm8(nc, ps, tA, tB,
                     lambda t, k, _m=mm: t[:, k % 4, ts(_m, 128)],
                     lambda t, k: xT[:, k, :])
                nc.vector.tensor_scalar(
                    hT[:, m, :], ps, b1t[:, m:m + 1], 0.0,
                    op0=ALU.add, op1=ALU.max)
        w2 = c.w2_d[l][e]
        for half in range(2):
            tA = _load_wh(c, w2, half * 512, 0)
            tB = _load_wh(c, w2, half * 512, 1)
            for tt in range(TT):
                ps = c.psA.tile([128, 512], F32, tag="psA")
                _mm8(nc, ps, tA, tB,
                     lambda t, k, _tt=tt: hT[:, k, ts(_tt, 128)],
                     lambda t, k: t[:, k % 4, :],
                     tail=(c.onesr, b2r[:, ds(half * 512, 512)]))
                dst = y[:, tt, ds(half * 512, 512)]
                if e == 0:
                    nc.vector.tensor_scalar_mul(dst, ps, coef[:, tt, 0:1])
                else:
                    nc.vector.scalar_tensor_tensor(
                        dst, ps, coef[:, tt, e:e + 1], dst,
                        op0=ALU.mult, op1=ALU.add)
    return y


def _attn_qkv(c, a, yT, kvin):
    nc = c.nc
    kv_k = kvin[0:KV_HALF].rearrange("(j t) -> j t", t=TC)
    kv_v = kvin[KV_HALF:].rearrange("(t d) -> t d", d=D)
    qkb = c.small.tile([128, 16], F32, tag="qkb")
    nc.sync.dma_start(qkb, c.qkb_d[a])
    vbr = c.rows.tile([1, D], F32R, tag="vbr")
    nc.sync.dma_start(vbr, c.vb_d[a].bitcast(F32R))
    qT = c.qtp.tile([128, KC, TC], F32R, tag="qT")
    for blk in range(4):
        tA = _load_wh(c, c.wqk_d[a], blk * 512, 0)
        tB = _load_wh(c, c.wqk_d[a], blk * 512, 1)
        for sub in range(4):
            jt = blk * 4 + sub
            ps = c.psA.tile([128, 512], F32, tag="psA")
            _mm8(nc, ps, tA, tB,
                 lambda t, k, _s=sub: t[:, k % 4, ts(_s, 128)],
                 lambda t, k: yT[:, k, :])
            if jt < 8:
                nc.vector.tensor_scalar(
                    qT[:, jt, :], ps, qkb[:, jt:jt + 1], None, op0=ALU.add)
            else:
                kev = c.kvp.tile([128, TC], F32R, tag="kvev")
                nc.vector.tensor_scalar(
                    kev, ps, qkb[:, jt:jt + 1], None, op0=ALU.add)
                nc.sync.dma_start(kv_k[ts(jt - 8, 128), :].bitcast(F32R), kev)
    for half in range(2):
        tA = _load_wh(c, c.wv_d[a], half * 512, 0)
        tB = _load_wh(c, c.wv_d[a], half * 512, 1)
        for tt in range(TT):
            ps = c.psA.tile([128, 512], F32, tag="psA")
            _mm8(nc, ps, tA, tB,
                 lambda t, k, _tt=tt: yT[:, k, ts(_tt, 128)],
                 lambda t, k: t[:, k % 4, :],
                 tail=(c.onesr, vbr[:, ds(half * 512, 512)]))
            vev = c.kvp.tile([128, TC], F32R, tag="kvev")
            nc.vector.tensor_copy(vev, ps)
            nc.sync.dma_start(
                kv_v[ts(tt, 128), ds(half * 512, 512)].bitcast(F32R), vev)
    return qT


def _attn_heads(c, qT, kvout):
    nc = c.nc
    oT = c.xtp.tile([128, KC, TC], F32R, tag="bigT")
    for h in range(H):
        av = c.psav.tile([128, 512], F32, tag="av")
        den = c.psden.tile([1, 512], F32, tag="den")
        nmm = 0
        for r in range(4):
            k_r = kvout[r, 0:KV_HALF].rearrange("(j t) -> j t", t=TC)
            v_r = kvout[r, KV_HALF:].rearrange(
                "(st p hh hd) -> p st hh hd", p=128, hh=H, hd=HD)
            kblk = c.kvp.tile([128, TC], F32R, tag="kvev")
            nc.sync.dma_start(kblk, k_r[ts(h, 128), :].bitcast(F32R))
            vblk = c.kvp.tile([128, 4, HD], F32R, tag="vblk")
            nc.sync.dma_start(vblk, v_r[:, :, h, :].bitcast(F32R))
            for pair in range(2):
                epst = c.pse.tile([128, 2, 512], F32, tag="eps")
                for sub in range(2):
                    st = pair * 2 + sub
                    nc.tensor.matmul(
                        epst[:, sub, :], lhsT=kblk[:, ts(st, 128)],
                        rhs=qT[:, h, :], start=True, stop=True)
                et = c.epool.tile([128, 2, 512], F32R, tag="et")
                nc.scalar.activation(et, epst, AF.Exp, scale=INV_SQRT_HD)
                for sub in range(2):
                    st = pair * 2 + sub
                    nmm += 1
                    nc.tensor.matmul(av, lhsT=vblk[:, st, :],
                                     rhs=et[:, sub, :],
                                     start=(nmm == 1), stop=(nmm == 16))
                    nc.tensor.matmul(den, lhsT=c.onesc, rhs=et[:, sub, :],
                                     start=(nmm == 1), stop=(nmm == 16))
        rcp = c.rows.tile([1, 512], F32R, tag="rcp")
        with nc.allow_low_precision(reason="f32r is a 4-byte container"):
            nc.vector.reciprocal(rcp, den)
        bc = c.psA.tile([128, 512], F32, tag="psA")
        nc.tensor.matmul(bc, lhsT=c.onesr, rhs=rcp, start=True, stop=True)
        bcs = c.kvp.tile([128, 512], F32, tag="bcs")
        nc.vector.tensor_copy(bcs, bc)
        nc.vector.tensor_tensor(oT[:, h, :], av, bcs, op=ALU.mult)
    return oT


def _attn_out_ln(c, a, oT, x_n):
    nc = c.nc
    obr = c.rows.tile([1, D], F32R, tag="obr")
    nc.sync.dma_start(obr, c.ob_d[a].bitcast(F32R))
    zn = c.natp.tile([128, TT, D], F32, tag="nat")
    for half in range(2):
        tA = _load_wh(c, c.wo_d[a], half * 512, 0)
        tB = _load_wh(c, c.wo_d[a], half * 512, 1)
        for tt in range(TT):
            ps = c.psA.tile([128, 512], F32, tag="psA")
            _mm8(nc, ps, tA, tB,
                 lambda t, k, _tt=tt: oT[:, k, ts(_tt, 128)],
                 lambda t, k: t[:, k % 4, :],
                 tail=(c.onesr, obr[:, ds(half * 512, 512)]))
            nc.vector.tensor_tensor(
                zn[:, tt, ds(half * 512, 512)], ps,
                x_n[:, tt, ds(half * 512, 512)], op=ALU.add)
    xnn = c.xnat.tile([128, TT, D], F32, tag="xnat")
    for tt in range(TT):
        sm = c.small.tile([128, 1], F32, tag="sm")
        nc.vector.reduce_sum(sm, zn[:, tt, :], axis=AX.X)
        negm = c.small.tile([128, 1], F32, tag="negm")
        nc.vector.tensor_scalar_mul(negm, sm, -1.0 / D)
        nc.vector.tensor_scalar(
            zn[:, tt, :], zn[:, tt, :], negm, None, op0=ALU.add)
        nc.vector.tensor_tensor(
            xnn[:, tt, :], zn[:, tt, :], zn[:, tt, :], op=ALU.mult)
        vs = c.small.tile([128, 1], F32, tag="vs")
        nc.vector.reduce_sum(vs, xnn[:, tt, :], axis=AX.X)
        sd = c.small.tile([128, 1], F32, tag="sd")
        nc.scalar.activation(sd, vs, AF.Sqrt, bias=c.epsb, scale=1.0 / D)
        rstd = c.small.tile([128, 1], F32, tag="rstd")
        nc.vector.reciprocal(rstd, sd)
        nc.vector.scalar_tensor_tensor(
            xnn[:, tt, :], zn[:, tt, :], rstd, c.lng,
            op0=ALU.mult, op1=ALU.mult)
        nc.vector.tensor_tensor(xnn[:, tt, :], xnn[:, tt, :], c.lnb,
                                op=ALU.add)
    return xnn


def _body(c):
    nc = c.nc
    xT, x_n = _input_proj(c)
    for l in range(L):
        coef = _router(c, l, xT)
        y = _moe_experts(c, l, xT, coef)
        if l == L - 1:
            for tt in range(TT):
                nc.sync.dma_start(c.y_out[ts(tt, 128), :], y[:, tt, :])
            break
        a = l
        yT = c.xtp.tile([128, KC, TC], F32R, tag="bigT")
        _transpose_nat_to_T(c, y, yT)
        kvin = c.dpool.tile([2 * KV_HALF], F32, tag="kvin")
        kvout = c.dpool.tile([4, 2 * KV_HALF], F32, tag="kvout")
        qT = _attn_qkv(c, a, yT, kvin)
        nc.gpsimd.collective_compute(
            "AllGather", ALU.bypass,
            ins=[kvin[:].opt()],
            outs=[kvout[:].opt()],
            replica_groups=GROUP,
        )
        oT = _attn_heads(c, qT, kvout)
        x_n = _attn_out_ln(c, a, oT, x_n)
        xT = c.xtp.tile([128, KC, TC], F32R, tag="bigT")
        _transpose_nat_to_T(c, x_n, xT)


def build_nc():
    nc = bacc.Bacc("TRN2", target_bir_lowering=False, debug=False,
                   num_devices=NC)
    c = _Ctx()
    c.nc = nc

    def din(name, shape):
        return nc.dram_tensor(name, shape, F32, kind="ExternalInput").ap()

    c.xt0_d = din("XT0", [D, TC])
    c.wi_d = din("WI", [D, D])
    c.ib_d = din("IB", [128, KC])
    c.lng_d = din("LNG", [128, D])
    c.lnb_d = din("LNB", [128, D])
    c.eye_d = din("EYE", [128, 128])
    c.onesr_d = din("ONESR", [1, 128])
    c.onesc_d = din("ONESC", [128, 1])
    c.rw_d = [din(f"RW{l}", [D, E]) for l in range(L)]
    c.rbb_d = [din(f"RBB{l}", [128, E]) for l in range(L)]
    c.w1_d = [din(f"W1_{l}", [E, D, D]) for l in range(L)]
    c.b1_d = [din(f"B1_{l}", [E, 128, KC]) for l in range(L)]
    c.w2_d = [din(f"W2_{l}", [E, D, D]) for l in range(L)]
    c.b2_d = [din(f"B2_{l}", [E, D]) for l in range(L)]
    c.wqk_d = [din(f"WQK{a}", [D, 2 * D]) for a in range(L - 1)]
    c.qkb_d = [din(f"QKB{a}", [128, 16]) for a in range(L - 1)]
    c.wv_d = [din(f"WV{a}", [D, D]) for a in range(L - 1)]
    c.vb_d = [din(f"VB{a}", [1, D]) for a in range(L - 1)]
    c.wo_d = [din(f"WO{a}", [D, D]) for a in range(L - 1)]
    c.ob_d = [din(f"OB{a}", [1, D]) for a in range(L - 1)]
    c.y_out = nc.dram_tensor("Y", [TC, D], F32, kind="ExternalOutput").ap()
    c.lg_out = nc.dram_tensor("LG", [L, TC, E], F32,
                              kind="ExternalOutput").ap()

    with ExitStack() as st:
        tc = st.enter_context(tile.TileContext(nc))
        pool = lambda name, bufs, **kw: st.enter_context(
            tc.tile_pool(name=name, bufs=bufs, **kw))
        cpool = pool("consts", 1)
        c.xtp = pool("xtp", 2)
        c.qtp = pool("qtp", 1)
        c.htp = pool("htp", 1)
        c.wexp = pool("wexp", 3)
        c.xnat = pool("xnat", 2)
        c.natp = pool("natp", 1)
        c.epool = pool("epool", 2)
        c.kvp = pool("kvp", 2)
        c.rows = pool("rows", 1)
        c.small = pool("small", 3)
        c.psA = pool("psA", 2, space="PSUM")
        c.pse = pool("pse", 2, space="PSUM")
        c.psav = pool("psav", 1, space="PSUM")
        c.psden = pool("psden", 1, space="PSUM")
        c.dpool = pool("dram", 2, space="DRAM")

        c.lng = cpool.tile([128, D], F32, tag="lng")
        nc.sync.dma_start(c.lng, c.lng_d)
        c.lnb = cpool.tile([128, D], F32, tag="lnb")
        nc.sync.dma_start(c.lnb, c.lnb_d)
        c.eye = cpool.tile([128, 128], F32, tag="eye")
        nc.sync.dma_start(c.eye, c.eye_d)
        c.eyer = cpool.tile([128, 128], F32R, tag="eyer")
        nc.sync.dma_start(c.eyer, c.eye_d.bitcast(F32R))
        c.onesr = cpool.tile([1, 128], F32R, tag="onesr")
        nc.sync.dma_start(c.onesr, c.onesr_d.bitcast(F32R))
        c.onesc = cpool.tile([128, 1], F32R, tag="onesc")
        nc.sync.dma_start(c.onesc, c.onesc_d.bitcast(F32R))
        c.ibt = cpool.tile([128, KC], F32, tag="ibt")
        nc.sync.dma_start(c.ibt, c.ib_d)
        c.epsb = cpool.tile([128, 1], F32, tag="epsb")
        nc.vector.memset(c.epsb, EPS)

        _body(c)

    nc.compile()
    return nc


def _host_prep(params):
    g = {}
    p = params
    asnp = lambda a: np.ascontiguousarray(np.asarray(a, dtype=np.float32))
    g["WI"] = asnp(np.asarray(p["inp_w"]).T)
    g["IB"] = asnp(np.asarray(p["inp_b"]).reshape(KC, 128).T)
    g["LNG"] = asnp(np.tile(np.asarray(p["ln_g"])[None, :], (128, 1)))
    g["LNB"] = asnp(np.tile(np.asarray(p["ln_b"])[None, :], (128, 1)))
    g["EYE"] = np.eye(128, dtype=np.float32)
    g["ONESR"] = np.ones((1, 128), np.float32)
    g["ONESC"] = np.ones((128, 1), np.float32)
    for l in range(L):
        lay = p["layers"][l]
        g[f"RW{l}"] = asnp(lay["router_w"])
        g[f"RBB{l}"] = asnp(np.tile(np.asarray(lay["router_b"])[None, :],
                                    (128, 1)))
        g[f"W1_{l}"] = asnp(lay["w1"])
        g[f"B1_{l}"] = asnp(np.asarray(lay["b1"]).reshape(E, KC, 128)
                            .transpose(0, 2, 1))
        g[f"W2_{l}"] = asnp(lay["w2"])
        g[f"B2_{l}"] = asnp(lay["b2"])
    for a in range(L - 1):
        att = p["attn"][a]
        qkv_w = np.asarray(att["qkv_w"])
        qkv_b = np.asarray(att["qkv_b"])
        g[f"WQK{a}"] = asnp(qkv_w[:2 * D].T)
        g[f"QKB{a}"] = asnp(qkv_b[:2 * D].reshape(16, 128).T)
        g[f"WV{a}"] = asnp(qkv_w[2 * D:].T)
        g[f"VB{a}"] = asnp(qkv_b[2 * D:][None, :])
        g[f"WO{a}"] = asnp(np.asarray(att["out_w"]).T)
        g[f"OB{a}"] = asnp(np.asarray(att["out_b"])[None, :])
    return g


def kernel(x, params, _trace=False):
    x = np.asarray(x, dtype=np.float32)
    if "nc" not in _CACHE:
        _CACHE["nc"] = build_nc()
    nc = _CACHE["nc"]
    shared = _host_prep(params)
    in_maps = []
    for core in range(NC):
        b, chunk = core // 4, core % 4
        m = dict(shared)
        m["XT0"] = np.ascontiguousarray(x[b, chunk * TC:(chunk + 1) * TC, :].T)
        in_maps.append(m)
    res = bass_utils.run_bass_kernel_spmd(
        nc, in_maps, core_ids=list(range(NC)), trace=_trace)
    x_out = np.zeros((B, S, D), np.float32)
    lg = np.zeros((L, B, S, E), np.float32)
    for core in range(NC):
        b, chunk = core // 4, core % 4
        sl = slice(chunk * TC, (chunk + 1) * TC)
        x_out[b, sl, :] = res.results[core]["Y"]
        lg[:, b, sl, :] = res.results[core]["LG"]
    if _trace:
        _CACHE["last_result"] = res
    return x_out, [lg[i] for i in range(L)]


# revision 18
# speedup vs baseline: 1.0819x; 1.0819x over previous
"""Trainium2 Bass kernel for nn_MoEModel_18476949307967.

4-layer MoE transformer: B=2, S=2048, D=1024, E=8 experts top-2, H=8 heads,
3 interleaved attention layers. 8-way data-parallel over tokens (4096 tokens
-> 512/core; cores 0-3 = batch 0, cores 4-7 = batch 1). Attention K/V is
all-gathered within each 4-core batch group. Dense all-expert FFN per core
with top-2 masked combine (matches the reference exactly). Matmuls run in
float32r (tf32-like, full PE rate); the residual stream stays fp32.

Self-contained: hardcodes all shapes; no imports from the problem dir.
"""
from contextlib import ExitStack

import numpy as np

import concourse.bass as bass
import concourse.mybir as mybir
import concourse.tile as tile
from concourse import bacc, bass_utils
from concourse.bass import ts, ds

F32 = mybir.dt.float32
# Matmul operand dtype: float32 is exact (matches the reference's expert
# top-2 selections bit-for-bit in practice) at 4 PE cycles/row; float32r
# (tf32-like) runs 4x faster but its ~2e-4 logit error flips a couple of
# near-tied top-2 router selections vs the fp32 reference.
F32R = mybir.dt.float32
AF = mybir.ActivationFunctionType
ALU = mybir.AluOpType
AX = mybir.AxisListType

B, S, D, E, K, L, H = 2, 2048, 1024, 8, 2, 4, 8
HD = D // H                       # 128
NC = 8                            # cores
TC = (B * S) // NC                # 512 tokens per core
TT = TC // 128                    # 4 token tiles
KC = D // 128                     # 8 contraction subtiles
GROUP = [[0, 1, 2, 3], [4, 5, 6, 7]]
INV_SQRT_HD = 1.0 / float(np.sqrt(HD))
EPS = 1e-5
KV_HALF = D * TC

_CACHE = {}


class _Ctx:
    pass


def _load_wh(c, w2d, col_off, kh):
    """[128, 4, 512] f32r tile <- w2d[kh*512:(kh+1)*512, col_off:col_off+512]."""
    t = c.wexp.tile([128, 4, 512], F32R, tag="wh")
    c.nc.sync.dma_start(
        t, w2d[ds(kh * 512, 512), ds(col_off, 512)]
        .rearrange("(kc p) f -> p kc f", p=128).bitcast(F32R))
    return t


def _mm8(nc, ps, tA, tB, lhs_fn, rhs_fn, tail=None):
    """ps = sum_{kc<8} lhs(t, kc).T @ rhs(kc), lhsT slices from tA/tB halves."""
    for kc in range(KC):
        t = tA if kc < 4 else tB
        last = (kc == KC - 1) and tail is None
        nc.tensor.matmul(ps, lhsT=lhs_fn(t, kc), rhs=rhs_fn(t, kc),
                         start=(kc == 0), stop=last)
    if tail is not None:
        nc.tensor.matmul(ps, lhsT=tail[0], rhs=tail[1], start=False, stop=True)


def _transpose_nat_to_T(c, src_nat, dst_T):
    nc = c.nc
    for m in range(KC):
        for tt in range(TT):
            pt = c.psA.tile([128, 512], F32, tag="psA")
            nc.tensor.transpose(pt[:, :128], src_nat[:, tt, ts(m, 128)], c.eye)
            nc.vector.tensor_copy(dst_T[:, m, ts(tt, 128)], pt[:, :128])


def _input_proj(c):
    nc = c.nc
    x0 = c.xtp.tile([128, KC, TC], F32R, tag="bigT")
    nc.sync.dma_start(
        x0, c.xt0_d.rearrange("(kc p) t -> p kc t", p=128).bitcast(F32R))
    xT = c.xtp.tile([128, KC, TC], F32R, tag="bigT")
    for half in range(2):
        tA = _load_wh(c, c.wi_d, half * 512, 0)
        tB = _load_wh(c, c.wi_d, half * 512, 1)
        for mm in range(4):
            m = half * 4 + mm
            ps = c.psA.tile([128, 512], F32, tag="psA")
            _mm8(nc, ps, tA, tB,
                 lambda t, k, _m=mm: t[:, k % 4, ts(_m, 128)],
                 lambda t, k: x0[:, k, :])
            nc.vector.tensor_scalar(
                xT[:, m, :], ps, c.ibt[:, m:m + 1], None, op0=ALU.add)
    x_n = c.xnat.tile([128, TT, D], F32, tag="xnat")
    for m in range(KC):
        for tt in range(TT):
            pt = c.psA.tile([128, 512], F32, tag="psA")
            nc.tensor.matmul(pt[:, :128], lhsT=xT[:, m, ts(tt, 128)],
                             rhs=c.eyer, start=True, stop=True)
            nc.vector.tensor_copy(x_n[:, tt, ts(m, 128)], pt[:, :128])
    return xT, x_n


def _router(c, l, xT):
    nc = c.nc
    rw = c.small.tile([128, KC, E], F32R, tag="rw")
    nc.sync.dma_start(
        rw, c.rw_d[l].rearrange("(kc p) e -> p kc e", p=128).bitcast(F32R))
    rbb = c.small.tile([128, E], F32, tag="rbb")
    nc.sync.dma_start(rbb, c.rbb_d[l])
    coef = c.small.tile([128, TT, E], F32, tag="coef")
    for tt in range(TT):
        psl = c.psA.tile([128, 512], F32, tag="psA")
        for k in range(KC):
            nc.tensor.matmul(psl[:, :E], lhsT=xT[:, k, ts(tt, 128)],
                             rhs=rw[:, k, :], start=(k == 0),
                             stop=(k == KC - 1))
        lgt = c.small.tile([128, E], F32, tag="lgt")
        nc.vector.tensor_tensor(lgt, psl[:, :E], rbb, op=ALU.add)
        nc.sync.dma_start(c.lg_out[l, ts(tt, 128), :], lgt)
        m8 = c.small.tile([128, 8], F32, tag="m8")
        nc.vector.max(m8, lgt)
        negmax = c.small.tile([128, 1], F32, tag="negmax")
        nc.vector.tensor_scalar_mul(negmax, m8[:, 0:1], -1.0)
        elg = c.small.tile([128, E], F32, tag="elg")
        denl = c.small.tile([128, 1], F32, tag="denl")
        nc.scalar.activation(elg, lgt, AF.Exp, bias=negmax, scale=1.0,
                             accum_out=denl)
        rcl = c.small.tile([128, 1], F32, tag="rcl")
        nc.vector.reciprocal(rcl, denl)
        probs = c.small.tile([128, E], F32, tag="probs")
        nc.vector.tensor_scalar_mul(probs, elg, rcl)
        nc.vector.scalar_tensor_tensor(
            coef[:, tt, :], lgt, m8[:, 1:2], probs,
            op0=ALU.is_ge, op1=ALU.mult)
    return coef


def _moe_experts(c, l, xT, coef):
    nc = c.nc
    y = c.natp.tile([128, TT, D], F32, tag="nat")
    # coefT [E, tok] for the single rank-8 bias matmul sum_e coef_e * b2_e
    coefT = c.small.tile([8, TT, 128], F32R, tag="coefT")
    for tt in range(TT):
        pt = c.psA.tile([128, 512], F32, tag="psA")
        nc.tensor.transpose(pt[:8, :128], coef[:, tt, :], c.eye)
        nc.vector.tensor_copy(coefT[:, tt, :], pt[:8, :128])
    b2all = c.small.tile([8, D], F32R, tag="b2all")
    nc.sync.dma_start(b2all, c.b2_d[l][:].bitcast(F32R))
    for e in range(E):
        b1t = c.small.tile([128, KC], F32, tag="b1t")
        nc.sync.dma_start(b1t, c.b1_d[l][e])
        hT = c.htp.tile([128, KC, TC], F32R, tag="hT")
        w1 = c.w1_d[l][e]
        for half in range(2):
            tA = _load_wh(c, w1, half * 512, 0)
            tB = _load_wh(c, w1, half * 512, 1)
            for mm in range(4):
                m = half * 4 + mm
                ps = c.psA.tile([128, 512], F32, tag="psA")
                _mm8(nc, ps, tA, tB,
                     lambda t, k, _m=mm: t[:, k % 4, ts(_m, 128)],
                     lambda t, k: xT[:, k, :])
                nc.vector.tensor_scalar(
                    hT[:, m, :], ps, b1t[:, m:m + 1], 0.0,
                    op0=ALU.add, op1=ALU.max)
        w2 = c.w2_d[l][e]
        for half in range(2):
            tA = _load_wh(c, w2, half * 512, 0)
            tB = _load_wh(c, w2, half * 512, 1)
            for tt in range(TT):
                ps = c.psA.tile([128, 512], F32, tag="psA")
                _mm8(nc, ps, tA, tB,
                     lambda t, k, _tt=tt: hT[:, k, ts(_tt, 128)],
                     lambda t, k: t[:, k % 4, :])
                dst = y[:, tt, ds(half * 512, 512)]
                if e == 0:
                    nc.vector.tensor_scalar_mul(dst, ps, coef[:, tt, 0:1])
                else:
                    nc.vector.scalar_tensor_tensor(
                        dst, ps, coef[:, tt, e:e + 1], dst,
                        op0=ALU.mult, op1=ALU.add)
    # y += coefT.T @ b2 (covers the per-expert b2 bias of every selected
    # expert in one rank-8 matmul per [tok, d-chunk] tile)
    for tt in range(TT):
        for ch in range(2):
            psb = c.psA.tile([128, 512], F32, tag="psA")
            nc.tensor.matmul(psb, lhsT=coefT[:, tt, :],
                             rhs=b2all[:, ds(ch * 512, 512)],
                             start=True, stop=True)
            nc.vector.tensor_tensor(
                y[:, tt, ds(ch * 512, 512)], y[:, tt, ds(ch * 512, 512)],
                psb, op=ALU.add)
    return y


def _attn_qkv(c, a, yT, kvin_k, kvin_v, kvout_k, kvout_v):
    """K projections first so AG(K) overlaps Q/V compute; AG(V) follows the
    V projections and overlaps the first heads' score/exp work."""
    nc = c.nc
    kv_k = kvin_k[:].rearrange("(j t) -> j t", t=TC)
    kv_v = kvin_v[:].rearrange("(t d) -> t d", d=D)
    qkb = c.small.tile([128, 16], F32, tag="qkb")
    nc.sync.dma_start(qkb, c.qkb_d[a])
    vbr = c.rows.tile([1, D], F32R, tag="vbr")
    nc.sync.dma_start(vbr, c.vb_d[a].bitcast(F32R))
    qT = c.qtp.tile([128, KC, TC], F32R, tag="qT")
    for blk in (2, 3, 0, 1):
        tA = _load_wh(c, c.wqk_d[a], blk * 512, 0)
        tB = _load_wh(c, c.wqk_d[a], blk * 512, 1)
        for sub in range(4):
            jt = blk * 4 + sub
            ps = c.psA.tile([128, 512], F32, tag="psA")
            _mm8(nc, ps, tA, tB,
                 lambda t, k, _s=sub: t[:, k % 4, ts(_s, 128)],
                 lambda t, k: yT[:, k, :])
            if jt < 8:
                nc.vector.tensor_scalar(
                    qT[:, jt, :], ps, qkb[:, jt:jt + 1], None, op0=ALU.add)
            else:
                kev = c.kvp.tile([128, TC], F32R, tag="kvev")
                nc.vector.tensor_scalar(
                    kev, ps, qkb[:, jt:jt + 1], None, op0=ALU.add)
                nc.sync.dma_start(kv_k[ts(jt - 8, 128), :].bitcast(F32R), kev)
    for half in range(2):
        tA = _load_wh(c, c.wv_d[a], half * 512, 0)
        tB = _load_wh(c, c.wv_d[a], half * 512, 1)
        for tt in range(TT):
            ps = c.psA.tile([128, 512], F32, tag="psA")
            _mm8(nc, ps, tA, tB,
                 lambda t, k, _tt=tt: yT[:, k, ts(_tt, 128)],
                 lambda t, k: t[:, k % 4, :],
                 tail=(c.onesr, vbr[:, ds(half * 512, 512)]))
            vev = c.kvp.tile([128, TC], F32R, tag="kvev")
            nc.vector.tensor_copy(vev, ps)
            nc.sync.dma_start(
                kv_v[ts(tt, 128), ds(half * 512, 512)].bitcast(F32R), vev)
    return qT


def _attn_heads(c, qT, kvout_k, kvout_v):
    nc = c.nc
    oT = c.xtp.tile([128, KC, TC], F32R, tag="bigT")
    for h in range(H):
        av = c.psav.tile([128, 512], F32, tag="av")
        den = c.psden.tile([1, 512], F32, tag="den")
        # e-tile sum on DVE so the softmax denominator needs one PE matmul
        eacc = c.kvp.tile([128, 512], F32, tag="eacc")
        nmm = 0
        for r in range(4):
            k_r = kvout_k[r].rearrange("(j t) -> j t", t=TC)
            v_r = kvout_v[r].rearrange(
                "(st p hh hd) -> p st hh hd", p=128, hh=H, hd=HD)
            kblk = c.kvp.tile([128, TC], F32R, tag="kvev")
            nc.sync.dma_start(kblk, k_r[ts(h, 128), :].bitcast(F32R))
            vblk = c.kvp.tile([128, 4, HD], F32R, tag="vblk")
            nc.sync.dma_start(vblk, v_r[:, :, h, :].bitcast(F32R))
            for pair in range(2):
                epst = c.pse.tile([128, 2, 512], F32, tag="eps")
                for sub in range(2):
                    st = pair * 2 + sub
                    nc.tensor.matmul(
                        epst[:, sub, :], lhsT=kblk[:, ts(st, 128)],
                        rhs=qT[:, h, :], start=True, stop=True)
                et = c.epool.tile([128, 2, 512], F32R, tag="et")
                nc.scalar.activation(et, epst, AF.Exp, scale=INV_SQRT_HD)
                if nmm == 0:
                    nc.vector.tensor_tensor(eacc, et[:, 0, :], et[:, 1, :],
                                            op=ALU.add)
                else:
                    nc.vector.tensor_tensor(eacc, eacc, et[:, 0, :],
                                            op=ALU.add)
                    nc.vector.tensor_tensor(eacc, eacc, et[:, 1, :],
                                            op=ALU.add)
                for sub in range(2):
                    st = pair * 2 + sub
                    nmm += 1
                    nc.tensor.matmul(av, lhsT=vblk[:, st, :],
                                     rhs=et[:, sub, :],
                                     start=(nmm == 1), stop=(nmm == 16))
        nc.tensor.matmul(den, lhsT=c.onesc, rhs=eacc.bitcast(F32R),
                         start=True, stop=True)
        rcp = c.rows.tile([1, 512], F32R, tag="rcp")
        with nc.allow_low_precision(reason="f32r is a 4-byte container"):
            nc.vector.reciprocal(rcp, den)
        bc = c.psA.tile([128, 512], F32, tag="psA")
        nc.tensor.matmul(bc, lhsT=c.onesr, rhs=rcp, start=True, stop=True)
        bcs = c.kvp.tile([128, 512], F32, tag="bcs")
        nc.vector.tensor_copy(bcs, bc)
        nc.vector.tensor_tensor(oT[:, h, :], av, bcs, op=ALU.mult)
    return oT


def _attn_out_ln(c, a, oT, x_n):
    nc = c.nc
    obr = c.rows.tile([1, D], F32R, tag="obr")
    nc.sync.dma_start(obr, c.ob_d[a].bitcast(F32R))
    zn = c.natp.tile([128, TT, D], F32, tag="nat")
    for half in range(2):
        tA = _load_wh(c, c.wo_d[a], half * 512, 0)
        tB = _load_wh(c, c.wo_d[a], half * 512, 1)
        for tt in range(TT):
            ps = c.psA.tile([128, 512], F32, tag="psA")
            _mm8(nc, ps, tA, tB,
                 lambda t, k, _tt=tt: oT[:, k, ts(_tt, 128)],
                 lambda t, k: t[:, k % 4, :],
                 tail=(c.onesr, obr[:, ds(half * 512, 512)]))
            nc.vector.tensor_tensor(
                zn[:, tt, ds(half * 512, 512)], ps,
                x_n[:, tt, ds(half * 512, 512)], op=ALU.add)
    xnn = c.xnat.tile([128, TT, D], F32, tag="xnat")
    for tt in range(TT):
        sm = c.small.tile([128, 1], F32, tag="sm")
        nc.vector.reduce_sum(sm, zn[:, tt, :], axis=AX.X)
        negm = c.small.tile([128, 1], F32, tag="negm")
        nc.vector.tensor_scalar_mul(negm, sm, -1.0 / D)
        nc.vector.tensor_scalar(
            zn[:, tt, :], zn[:, tt, :], negm, None, op0=ALU.add)
        nc.vector.tensor_tensor(
            xnn[:, tt, :], zn[:, tt, :], zn[:, tt, :], op=ALU.mult)
        vs = c.small.tile([128, 1], F32, tag="vs")
        nc.vector.reduce_sum(vs, xnn[:, tt, :], axis=AX.X)
        sd = c.small.tile([128, 1], F32, tag="sd")
        nc.scalar.activation(sd, vs, AF.Sqrt, bias=c.epsb, scale=1.0 / D)
        rstd = c.small.tile([128, 1], F32, tag="rstd")
        nc.vector.reciprocal(rstd, sd)
        nc.vector.scalar_tensor_tensor(
            xnn[:, tt, :], zn[:, tt, :], rstd, c.lng,
            op0=ALU.mult, op1=ALU.mult)
        nc.vector.tensor_tensor(xnn[:, tt, :], xnn[:, tt, :], c.lnb,
                                op=ALU.add)
    return xnn


def _body(c):
    nc = c.nc
    xT, x_n = _input_proj(c)
    for l in range(L):
        coef = _router(c, l, xT)
        y = _moe_experts(c, l, xT, coef)
        if l == L - 1:
            for tt in range(TT):
                nc.sync.dma_start(c.y_out[ts(tt, 128), :], y[:, tt, :])
            break
        a = l
        yT = c.xtp.tile([128, KC, TC], F32R, tag="bigT")
        _transpose_nat_to_T(c, y, yT)
        kvin = c.dpool.tile([2 * KV_HALF], F32, tag="kvin")
        kvout = c.dpool.tile([4, 2 * KV_HALF], F32, tag="kvout")
        qT = _attn_qkv(c, a, yT, kvin[0:KV_HALF], kvin[KV_HALF:],
                       None, None)
        nc.gpsimd.collective_compute(
            "AllGather", ALU.bypass,
            ins=[kvin[:].opt()], outs=[kvout[:].opt()],
            replica_groups=GROUP)
        oT = _attn_heads(c, qT,
                         [kvout[r, 0:KV_HALF] for r in range(4)],
                         [kvout[r, KV_HALF:] for r in range(4)])
        x_n = _attn_out_ln(c, a, oT, x_n)
        xT = c.xtp.tile([128, KC, TC], F32R, tag="bigT")
        _transpose_nat_to_T(c, x_n, xT)


def build_nc():
    nc = bacc.Bacc("TRN2", target_bir_lowering=False, debug=False,
                   num_devices=NC)
    c = _Ctx()
    c.nc = nc

    def din(name, shape):
        return nc.dram_tensor(name, shape, F32, kind="ExternalInput").ap()

    c.xt0_d = din("XT0", [D, TC])
    c.wi_d = din("WI", [D, D])
    c.ib_d = din("IB", [128, KC])
    c.lng_d = din("LNG", [128, D])
    c.lnb_d = din("LNB", [128, D])
    c.eye_d = din("EYE", [128, 128])
    c.onesr_d = din("ONESR", [1, 128])
    c.onesc_d = din("ONESC", [128, 1])
    c.rw_d = [din(f"RW{l}", [D, E]) for l in range(L)]
    c.rbb_d = [din(f"RBB{l}", [128, E]) for l in range(L)]
    c.w1_d = [din(f"W1_{l}", [E, D, D]) for l in range(L)]
    c.b1_d = [din(f"B1_{l}", [E, 128, KC]) for l in range(L)]
    c.w2_d = [din(f"W2_{l}", [E, D, D]) for l in range(L)]
    c.b2_d = [din(f"B2_{l}", [E, D]) for l in range(L)]
    c.wqk_d = [din(f"WQK{a}", [D, 2 * D]) for a in range(L - 1)]
    c.qkb_d = [din(f"QKB{a}", [128, 16]) for a in range(L - 1)]
    c.wv_d = [din(f"WV{a}", [D, D]) for a in range(L - 1)]
    c.vb_d = [din(f"VB{a}", [1, D]) for a in range(L - 1)]
    c.wo_d = [din(f"WO{a}", [D, D]) for a in range(L - 1)]
    c.ob_d = [din(f"OB{a}", [1, D]) for a in range(L - 1)]
    c.y_out = nc.dram_tensor("Y", [TC, D], F32, kind="ExternalOutput").ap()
    c.lg_out = nc.dram_tensor("LG", [L, TC, E], F32,
                              kind="ExternalOutput").ap()

    with ExitStack() as st:
        tc = st.enter_context(tile.TileContext(nc))
        pool = lambda name, bufs, **kw: st.enter_context(
            tc.tile_pool(name=name, bufs=bufs, **kw))
        cpool = pool("consts", 1)
        c.xtp = pool("xtp", 2)
        c.qtp = pool("qtp", 1)
        c.htp = pool("htp", 1)
        c.wexp = pool("wexp", 3)
        c.xnat = pool("xnat", 2)
        c.natp = pool("natp", 1)
        c.epool = pool("epool", 2)
        c.kvp = pool("kvp", 2)
        c.rows = pool("rows", 1)
        c.small = pool("small", 3)
        c.psA = pool("psA", 2, space="PSUM")
        c.pse = pool("pse", 2, space="PSUM")
        c.psav = pool("psav", 1, space="PSUM")
        c.psden = pool("psden", 1, space="PSUM")
        c.dpool = pool("dram", 2, space="DRAM")

        c.lng = cpool.tile([128, D], F32, tag="lng")
        nc.sync.dma_start(c.lng, c.lng_d)
        c.lnb = cpool.tile([128, D], F32, tag="lnb")
        nc.sync.dma_start(c.lnb, c.lnb_d)
        c.eye = cpool.tile([128, 128], F32, tag="eye")
        nc.sync.dma_start(c.eye, c.eye_d)
        c.eyer = cpool.tile([128, 128], F32R, tag="eyer")
        nc.sync.dma_start(c.eyer, c.eye_d.bitcast(F32R))
        c.onesr = cpool.tile([1, 128], F32R, tag="onesr")
        nc.sync.dma_start(c.onesr, c.onesr_d.bitcast(F32R))
        c.onesc = cpool.tile([128, 1], F32R, tag="onesc")
        nc.sync.dma_start(c.onesc, c.onesc_d.bitcast(F32R))
        c.ibt = cpool.tile([128, KC], F32, tag="ibt")
        nc.sync.dma_start(c.ibt, c.ib_d)
        c.epsb = cpool.tile([128, 1], F32, tag="epsb")
        nc.vector.memset(c.epsb, EPS)

        _body(c)

    nc.compile()
    return nc


def _host_prep(params):
    g = {}
    p = params
    asnp = lambda a: np.ascontiguousarray(np.asarray(a, dtype=np.float32))
    g["WI"] = asnp(np.asarray(p["inp_w"]).T)
    g["IB"] = asnp(np.asarray(p["inp_b"]).reshape(KC, 128).T)
    g["LNG"] = asnp(np.tile(np.asarray(p["ln_g"])[None, :], (128, 1)))
    g["LNB"] = asnp(np.tile(np.asarray(p["ln_b"])[None, :], (128, 1)))
    g["EYE"] = np.eye(128, dtype=np.float32)
    g["ONESR"] = np.ones((1, 128), np.float32)
    g["ONESC"] = np.ones((128, 1), np.float32)
    for l in range(L):
        lay = p["layers"][l]
        g[f"RW{l}"] = asnp(lay["router_w"])
        g[f"RBB{l}"] = asnp(np.tile(np.asarray(lay["router_b"])[None, :],
                                    (128, 1)))
        g[f"W1_{l}"] = asnp(lay["w1"])
        g[f"B1_{l}"] = asnp(np.asarray(lay["b1"]).reshape(E, KC, 128)
                            .transpose(0, 2, 1))
        g[f"W2_{l}"] = asnp(lay["w2"])
        g[f"B2_{l}"] = asnp(lay["b2"])
    for a in range(L - 1):
        att = p["attn"][a]
        qkv_w = np.asarray(att["qkv_w"])
        qkv_b = np.asarray(att["qkv_b"])
        g[f"WQK{a}"] = asnp(qkv_w[:2 * D].T)
        g[f"QKB{a}"] = asnp(qkv_b[:2 * D].reshape(16, 128).T)
        g[f"WV{a}"] = asnp(qkv_w[2 * D:].T)
        g[f"VB{a}"] = asnp(qkv_b[2 * D:][None, :])
        g[f"WO{a}"] = asnp(np.asarray(att["out_w"]).T)
        g[f"OB{a}"] = asnp(np.asarray(att["out_b"])[None, :])
    return g


def kernel(x, params, _trace=False):
    x = np.asarray(x, dtype=np.float32)
    if "nc" not in _CACHE:
        _CACHE["nc"] = build_nc()
    nc = _CACHE["nc"]
    shared = _host_prep(params)
    in_maps = []
    for core in range(NC):
        b, chunk = core // 4, core % 4
        m = dict(shared)
        m["XT0"] = np.ascontiguousarray(x[b, chunk * TC:(chunk + 1) * TC, :].T)
        in_maps.append(m)
    res = bass_utils.run_bass_kernel_spmd(
        nc, in_maps, core_ids=list(range(NC)), trace=_trace)
    x_out = np.zeros((B, S, D), np.float32)
    lg = np.zeros((L, B, S, E), np.float32)
    for core in range(NC):
        b, chunk = core // 4, core % 4
        sl = slice(chunk * TC, (chunk + 1) * TC)
        x_out[b, sl, :] = res.results[core]["Y"]
        lg[:, b, sl, :] = res.results[core]["LG"]
    if _trace:
        _CACHE["last_result"] = res
    return x_out, [lg[i] for i in range(L)]


# revision 19
# speedup vs baseline: 1.2299x; 1.1367x over previous
"""Trainium2 Bass kernel for nn_MoEModel_18476949307967.

4-layer MoE transformer: B=2, S=2048, D=1024, E=8 experts top-2, H=8 heads,
3 interleaved attention layers. 8-way data-parallel over tokens (4096 tokens
-> 512/core; cores 0-3 = batch 0, cores 4-7 = batch 1). Attention K/V is
all-gathered within each 4-core batch group. Dense all-expert FFN per core
with top-2 masked combine (matches the reference exactly). Matmuls run in
float32r (tf32-like, full PE rate); the residual stream stays fp32.

Self-contained: hardcodes all shapes; no imports from the problem dir.
"""
from contextlib import ExitStack

import numpy as np

import concourse.bass as bass
import concourse.mybir as mybir
import concourse.tile as tile
from concourse import bacc, bass_utils
from concourse.bass import ts, ds

F32 = mybir.dt.float32
BF16 = mybir.dt.bfloat16
# Matmul operand dtype: float32 is exact (matches the reference's expert
# top-2 selections bit-for-bit in practice) at 4 PE cycles/row; float32r
# (tf32-like) runs 4x faster but its ~2e-4 logit error flips a couple of
# near-tied top-2 router selections vs the fp32 reference.
F32R = mybir.dt.float32
AF = mybir.ActivationFunctionType
ALU = mybir.AluOpType
AX = mybir.AxisListType

B, S, D, E, K, L, H = 2, 2048, 1024, 8, 2, 4, 8
HD = D // H                       # 128
NC = 8                            # cores
TC = (B * S) // NC                # 512 tokens per core
TT = TC // 128                    # 4 token tiles
KC = D // 128                     # 8 contraction subtiles
GROUP = [[0, 1, 2, 3], [4, 5, 6, 7]]
INV_SQRT_HD = 1.0 / float(np.sqrt(HD))
EPS = 1e-5
KV_HALF = D * TC

_CACHE = {}


class _Ctx:
    pass


def _load_wh(c, w2d, col_off, kh):
    """[128, 4, 512] f32r tile <- w2d[kh*512:(kh+1)*512, col_off:col_off+512]."""
    t = c.wexp.tile([128, 4, 512], F32R, tag="wh")
    c.nc.sync.dma_start(
        t, w2d[ds(kh * 512, 512), ds(col_off, 512)]
        .rearrange("(kc p) f -> p kc f", p=128).bitcast(F32R))
    return t


def _load_bf(c, wh_d, wl_d, col_off, kh):
    """One [128, 4, 2, 512] bf16 tile: [:, :, 0, :] = hi, [:, :, 1, :] = lo."""
    t = c.wexp.tile([128, 4, 2, 512], BF16, tag="wh")
    sl = lambda d: d[ds(kh * 512, 512), ds(col_off, 512)].rearrange(
        "(kc p) f -> p kc f", p=128)
    c.nc.sync.dma_start(t[:, :, 0, :], sl(wh_d))
    c.nc.sync.dma_start(t[:, :, 1, :], sl(wl_d))
    return t


def _mm24(nc, ps, tA, tB, lhs_fn, rhs_fn):
    """ps = sum over 3 bf16 hi/lo product terms x 8 kc subtiles.
    lhs_fn(t, kc, hl) / rhs_fn(t, kc, hl) select the bf16 operand slices."""
    n = 0
    for lhl, rhl in ((0, 0), (0, 1), (1, 0)):
        for kc in range(KC):
            t = tA if kc < 4 else tB
            n += 1
            nc.tensor.matmul(ps, lhsT=lhs_fn(t, kc, lhl),
                             rhs=rhs_fn(t, kc, rhl),
                             start=(n == 1), stop=(n == 24))


def _mm8(nc, ps, tA, tB, lhs_fn, rhs_fn, tail=None):
    """ps = sum_{kc<8} lhs(t, kc).T @ rhs(kc), lhsT slices from tA/tB halves."""
    for kc in range(KC):
        t = tA if kc < 4 else tB
        last = (kc == KC - 1) and tail is None
        nc.tensor.matmul(ps, lhsT=lhs_fn(t, kc), rhs=rhs_fn(t, kc),
                         start=(kc == 0), stop=last)
    if tail is not None:
        nc.tensor.matmul(ps, lhsT=tail[0], rhs=tail[1], start=False, stop=True)


def _transpose_nat_to_T(c, src_nat, dst_T):
    nc = c.nc
    for m in range(KC):
        for tt in range(TT):
            pt = c.psA.tile([128, 512], F32, tag="psA")
            nc.tensor.transpose(pt[:, :128], src_nat[:, tt, ts(m, 128)], c.eye)
            nc.vector.tensor_copy(dst_T[:, m, ts(tt, 128)], pt[:, :128])


def _input_proj(c):
    nc = c.nc
    x0 = c.xtp.tile([128, KC, TC], F32R, tag="bigT")
    nc.sync.dma_start(
        x0, c.xt0_d.rearrange("(kc p) t -> p kc t", p=128).bitcast(F32R))
    xT = c.xtp.tile([128, KC, TC], F32R, tag="bigT")
    for half in range(2):
        tA = _load_wh(c, c.wi_d, half * 512, 0)
        tB = _load_wh(c, c.wi_d, half * 512, 1)
        for mm in range(4):
            m = half * 4 + mm
            ps = c.psA.tile([128, 512], F32, tag="psA")
            _mm8(nc, ps, tA, tB,
                 lambda t, k, _m=mm: t[:, k % 4, ts(_m, 128)],
                 lambda t, k: x0[:, k, :])
            nc.vector.tensor_scalar(
                xT[:, m, :], ps, c.ibt[:, m:m + 1], None, op0=ALU.add)
    x_n = c.xnat.tile([128, TT, D], F32, tag="xnat")
    for m in range(KC):
        for tt in range(TT):
            pt = c.psA.tile([128, 512], F32, tag="psA")
            nc.tensor.matmul(pt[:, :128], lhsT=xT[:, m, ts(tt, 128)],
                             rhs=c.eyer, start=True, stop=True)
            nc.vector.tensor_copy(x_n[:, tt, ts(m, 128)], pt[:, :128])
    return xT, x_n


def _router(c, l, xT):
    nc = c.nc
    rw = c.small.tile([128, KC, E], F32R, tag="rw")
    nc.sync.dma_start(
        rw, c.rw_d[l].rearrange("(kc p) e -> p kc e", p=128).bitcast(F32R))
    rbb = c.small.tile([128, E], F32, tag="rbb")
    nc.sync.dma_start(rbb, c.rbb_d[l])
    coef = c.small.tile([128, TT, E], F32, tag="coef")
    for tt in range(TT):
        psl = c.psA.tile([128, 512], F32, tag="psA")
        for k in range(KC):
            nc.tensor.matmul(psl[:, :E], lhsT=xT[:, k, ts(tt, 128)],
                             rhs=rw[:, k, :], start=(k == 0),
                             stop=(k == KC - 1))
        lgt = c.small.tile([128, E], F32, tag="lgt")
        nc.vector.tensor_tensor(lgt, psl[:, :E], rbb, op=ALU.add)
        nc.sync.dma_start(c.lg_out[l, ts(tt, 128), :], lgt)
        m8 = c.small.tile([128, 8], F32, tag="m8")
        nc.vector.max(m8, lgt)
        negmax = c.small.tile([128, 1], F32, tag="negmax")
        nc.vector.tensor_scalar_mul(negmax, m8[:, 0:1], -1.0)
        elg = c.small.tile([128, E], F32, tag="elg")
        denl = c.small.tile([128, 1], F32, tag="denl")
        nc.scalar.activation(elg, lgt, AF.Exp, bias=negmax, scale=1.0,
                             accum_out=denl)
        rcl = c.small.tile([128, 1], F32, tag="rcl")
        nc.vector.reciprocal(rcl, denl)
        probs = c.small.tile([128, E], F32, tag="probs")
        nc.vector.tensor_scalar_mul(probs, elg, rcl)
        nc.vector.scalar_tensor_tensor(
            coef[:, tt, :], lgt, m8[:, 1:2], probs,
            op0=ALU.is_ge, op1=ALU.mult)
    return coef


def _moe_experts(c, l, xT, coef):
    nc = c.nc
    y = c.natp.tile([128, TT, D], F32, tag="nat")
    # coefT [E, tok] for the single rank-8 bias matmul sum_e coef_e * b2_e
    coefT = c.rows.tile([8, TT, 128], F32R, tag="coefT")
    for tt in range(TT):
        pt = c.psA.tile([128, 512], F32, tag="psA")
        nc.tensor.transpose(pt[:8, :128], coef[:, tt, :], c.eye)
        nc.vector.tensor_copy(coefT[:, tt, :], pt[:8, :128])
    b2all = c.rows.tile([8, D], F32R, tag="b2all")
    nc.sync.dma_start(b2all, c.b2_d[l][:].bitcast(F32R))
    # split activations into exact bf16 hi/lo: x = xh + xl + O(2^-16)
    xbf = c.xtp.tile([128, KC, 2, TC], BF16, tag="bigT")
    for k in range(KC):
        nc.vector.tensor_copy(xbf[:, k, 0, :], xT[:, k, :])
        nc.vector.tensor_tensor(xbf[:, k, 1, :], xT[:, k, :],
                                xbf[:, k, 0, :], op=ALU.subtract)
    for e in range(E):
        b1t = c.small.tile([128, KC], F32, tag="b1t")
        nc.sync.dma_start(b1t, c.b1_d[l][e])
        hbf = c.htp.tile([128, KC, 2, TC], BF16, tag="hT")
        for half in range(2):
            tA = _load_bf(c, c.w1h_d[l][e], c.w1l_d[l][e], half * 512, 0)
            tB = _load_bf(c, c.w1h_d[l][e], c.w1l_d[l][e], half * 512, 1)
            for mm in range(4):
                m = half * 4 + mm
                ps = c.psA.tile([128, 512], F32, tag="psA")
                _mm24(nc, ps, tA, tB,
                      lambda t, k, hl, _m=mm: t[:, k % 4, hl, ts(_m, 128)],
                      lambda t, k, hl: xbf[:, k, hl, :])
                hrelu = c.kvp.tile([128, 512], F32, tag="hrelu")
                nc.vector.tensor_scalar(
                    hrelu, ps, b1t[:, m:m + 1], 0.0,
                    op0=ALU.add, op1=ALU.max)
                nc.vector.tensor_copy(hbf[:, m, 0, :], hrelu)
                nc.vector.tensor_tensor(hbf[:, m, 1, :], hrelu,
                                        hbf[:, m, 0, :], op=ALU.subtract)
        for half in range(2):
            tA = _load_bf(c, c.w2h_d[l][e], c.w2l_d[l][e], half * 512, 0)
            tB = _load_bf(c, c.w2h_d[l][e], c.w2l_d[l][e], half * 512, 1)
            for tt in range(TT):
                ps = c.psA.tile([128, 512], F32, tag="psA")
                _mm24(nc, ps, tA, tB,
                      lambda t, k, hl, _tt=tt: hbf[:, k, hl, ts(_tt, 128)],
                      lambda t, k, hl: t[:, k % 4, hl, :])
                dst = y[:, tt, ds(half * 512, 512)]
                if e == 0:
                    nc.vector.tensor_scalar_mul(dst, ps, coef[:, tt, 0:1])
                else:
                    nc.vector.scalar_tensor_tensor(
                        dst, ps, coef[:, tt, e:e + 1], dst,
                        op0=ALU.mult, op1=ALU.add)
    # y += coefT.T @ b2 (covers the per-expert b2 bias of every selected
    # expert in one rank-8 matmul per [tok, d-chunk] tile)
    for tt in range(TT):
        for ch in range(2):
            psb = c.psA.tile([128, 512], F32, tag="psA")
            nc.tensor.matmul(psb, lhsT=coefT[:, tt, :],
                             rhs=b2all[:, ds(ch * 512, 512)],
                             start=True, stop=True)
            nc.vector.tensor_tensor(
                y[:, tt, ds(ch * 512, 512)], y[:, tt, ds(ch * 512, 512)],
                psb, op=ALU.add)
    return y


def _attn_qkv(c, a, yT, kvin_k, kvin_v, kvout_k, kvout_v):
    """K projections first so AG(K) overlaps Q/V compute; AG(V) follows the
    V projections and overlaps the first heads' score/exp work."""
    nc = c.nc
    kv_k = kvin_k[:].rearrange("(j t) -> j t", t=TC)
    kv_v = kvin_v[:].rearrange("(t d) -> t d", d=D)
    qkb = c.small.tile([128, 16], F32, tag="qkb")
    nc.sync.dma_start(qkb, c.qkb_d[a])
    vbr = c.rows.tile([1, D], F32R, tag="vbr")
    nc.sync.dma_start(vbr, c.vb_d[a].bitcast(F32R))
    qT = c.qtp.tile([128, KC, TC], F32R, tag="qT")
    for blk in (2, 3, 0, 1):
        tA = _load_wh(c, c.wqk_d[a], blk * 512, 0)
        tB = _load_wh(c, c.wqk_d[a], blk * 512, 1)
        for sub in range(4):
            jt = blk * 4 + sub
            ps = c.psA.tile([128, 512], F32, tag="psA")
            _mm8(nc, ps, tA, tB,
                 lambda t, k, _s=sub: t[:, k % 4, ts(_s, 128)],
                 lambda t, k: yT[:, k, :])
            if jt < 8:
                nc.vector.tensor_scalar(
                    qT[:, jt, :], ps, qkb[:, jt:jt + 1], None, op0=ALU.add)
            else:
                kev = c.kvp.tile([128, TC], F32R, tag="kvev")
                nc.vector.tensor_scalar(
                    kev, ps, qkb[:, jt:jt + 1], None, op0=ALU.add)
                nc.sync.dma_start(kv_k[ts(jt - 8, 128), :].bitcast(F32R), kev)
    for half in range(2):
        tA = _load_wh(c, c.wv_d[a], half * 512, 0)
        tB = _load_wh(c, c.wv_d[a], half * 512, 1)
        for tt in range(TT):
            ps = c.psA.tile([128, 512], F32, tag="psA")
            _mm8(nc, ps, tA, tB,
                 lambda t, k, _tt=tt: yT[:, k, ts(_tt, 128)],
                 lambda t, k: t[:, k % 4, :],
                 tail=(c.onesr, vbr[:, ds(half * 512, 512)]))
            vev = c.kvp.tile([128, TC], F32R, tag="kvev")
            nc.vector.tensor_copy(vev, ps)
            nc.sync.dma_start(
                kv_v[ts(tt, 128), ds(half * 512, 512)].bitcast(F32R), vev)
    return qT


def _attn_heads(c, qT, kvout_k, kvout_v):
    nc = c.nc
    oT = c.xtp.tile([128, KC, TC], F32R, tag="bigT")
    for h in range(H):
        av = c.psav.tile([128, 512], F32, tag="av")
        den = c.psden.tile([1, 512], F32, tag="den")
        # e-tile sum on DVE so the softmax denominator needs one PE matmul
        eacc = c.kvp.tile([128, 512], F32, tag="eacc")
        nmm = 0
        for r in range(4):
            k_r = kvout_k[r].rearrange("(j t) -> j t", t=TC)
            v_r = kvout_v[r].rearrange(
                "(st p hh hd) -> p st hh hd", p=128, hh=H, hd=HD)
            kblk = c.kvp.tile([128, TC], F32R, tag="kvev")
            nc.sync.dma_start(kblk, k_r[ts(h, 128), :].bitcast(F32R))
            vblk = c.kvp.tile([128, 4, HD], F32R, tag="vblk")
            nc.sync.dma_start(vblk, v_r[:, :, h, :].bitcast(F32R))
            for pair in range(2):
                epst = c.pse.tile([128, 2, 512], F32, tag="eps")
                for sub in range(2):
                    st = pair * 2 + sub
                    nc.tensor.matmul(
                        epst[:, sub, :], lhsT=kblk[:, ts(st, 128)],
                        rhs=qT[:, h, :], start=True, stop=True)
                et = c.epool.tile([128, 2, 512], F32R, tag="et")
                nc.scalar.activation(et, epst, AF.Exp, scale=INV_SQRT_HD)
                if nmm == 0:
                    nc.vector.tensor_tensor(eacc, et[:, 0, :], et[:, 1, :],
                                            op=ALU.add)
                else:
                    nc.vector.tensor_tensor(eacc, eacc, et[:, 0, :],
                                            op=ALU.add)
                    nc.vector.tensor_tensor(eacc, eacc, et[:, 1, :],
                                            op=ALU.add)
                for sub in range(2):
                    st = pair * 2 + sub
                    nmm += 1
                    nc.tensor.matmul(av, lhsT=vblk[:, st, :],
                                     rhs=et[:, sub, :],
                                     start=(nmm == 1), stop=(nmm == 16))
        nc.tensor.matmul(den, lhsT=c.onesc, rhs=eacc.bitcast(F32R),
                         start=True, stop=True)
        rcp = c.rows.tile([1, 512], F32R, tag="rcp")
        with nc.allow_low_precision(reason="f32r is a 4-byte container"):
            nc.vector.reciprocal(rcp, den)
        bc = c.psA.tile([128, 512], F32, tag="psA")
        nc.tensor.matmul(bc, lhsT=c.onesr, rhs=rcp, start=True, stop=True)
        bcs = c.kvp.tile([128, 512], F32, tag="bcs")
        nc.vector.tensor_copy(bcs, bc)
        nc.vector.tensor_tensor(oT[:, h, :], av, bcs, op=ALU.mult)
    return oT


def _attn_out_ln(c, a, oT, x_n):
    nc = c.nc
    obr = c.rows.tile([1, D], F32R, tag="obr")
    nc.sync.dma_start(obr, c.ob_d[a].bitcast(F32R))
    zn = c.natp.tile([128, TT, D], F32, tag="nat")
    for half in range(2):
        tA = _load_wh(c, c.wo_d[a], half * 512, 0)
        tB = _load_wh(c, c.wo_d[a], half * 512, 1)
        for tt in range(TT):
            ps = c.psA.tile([128, 512], F32, tag="psA")
            _mm8(nc, ps, tA, tB,
                 lambda t, k, _tt=tt: oT[:, k, ts(_tt, 128)],
                 lambda t, k: t[:, k % 4, :],
                 tail=(c.onesr, obr[:, ds(half * 512, 512)]))
            nc.vector.tensor_tensor(
                zn[:, tt, ds(half * 512, 512)], ps,
                x_n[:, tt, ds(half * 512, 512)], op=ALU.add)
    xnn = c.xnat.tile([128, TT, D], F32, tag="xnat")
    for tt in range(TT):
        sm = c.small.tile([128, 1], F32, tag="sm")
        nc.vector.reduce_sum(sm, zn[:, tt, :], axis=AX.X)
        negm = c.small.tile([128, 1], F32, tag="negm")
        nc.vector.tensor_scalar_mul(negm, sm, -1.0 / D)
        nc.vector.tensor_scalar(
            zn[:, tt, :], zn[:, tt, :], negm, None, op0=ALU.add)
        nc.vector.tensor_tensor(
            xnn[:, tt, :], zn[:, tt, :], zn[:, tt, :], op=ALU.mult)
        vs = c.small.tile([128, 1], F32, tag="vs")
        nc.vector.reduce_sum(vs, xnn[:, tt, :], axis=AX.X)
        sd = c.small.tile([128, 1], F32, tag="sd")
        nc.scalar.activation(sd, vs, AF.Sqrt, bias=c.epsb, scale=1.0 / D)
        rstd = c.small.tile([128, 1], F32, tag="rstd")
        nc.vector.reciprocal(rstd, sd)
        nc.vector.scalar_tensor_tensor(
            xnn[:, tt, :], zn[:, tt, :], rstd, c.lng,
            op0=ALU.mult, op1=ALU.mult)
        nc.vector.tensor_tensor(xnn[:, tt, :], xnn[:, tt, :], c.lnb,
                                op=ALU.add)
    return xnn


def _body(c):
    nc = c.nc
    xT, x_n = _input_proj(c)
    for l in range(L):
        coef = _router(c, l, xT)
        y = _moe_experts(c, l, xT, coef)
        if l == L - 1:
            for tt in range(TT):
                nc.sync.dma_start(c.y_out[ts(tt, 128), :], y[:, tt, :])
            break
        a = l
        yT = c.xtp.tile([128, KC, TC], F32R, tag="bigT")
        _transpose_nat_to_T(c, y, yT)
        kvin = c.dpool.tile([2 * KV_HALF], F32, tag="kvin")
        kvout = c.dpool.tile([4, 2 * KV_HALF], F32, tag="kvout")
        qT = _attn_qkv(c, a, yT, kvin[0:KV_HALF], kvin[KV_HALF:],
                       None, None)
        nc.gpsimd.collective_compute(
            "AllGather", ALU.bypass,
            ins=[kvin[:].opt()], outs=[kvout[:].opt()],
            replica_groups=GROUP)
        oT = _attn_heads(c, qT,
                         [kvout[r, 0:KV_HALF] for r in range(4)],
                         [kvout[r, KV_HALF:] for r in range(4)])
        x_n = _attn_out_ln(c, a, oT, x_n)
        xT = c.xtp.tile([128, KC, TC], F32R, tag="bigT")
        _transpose_nat_to_T(c, x_n, xT)


def build_nc():
    nc = bacc.Bacc("TRN2", target_bir_lowering=False, debug=False,
                   num_devices=NC)
    c = _Ctx()
    c.nc = nc

    def din(name, shape):
        return nc.dram_tensor(name, shape, F32, kind="ExternalInput").ap()

    c.xt0_d = din("XT0", [D, TC])
    c.wi_d = din("WI", [D, D])
    c.ib_d = din("IB", [128, KC])
    c.lng_d = din("LNG", [128, D])
    c.lnb_d = din("LNB", [128, D])
    c.eye_d = din("EYE", [128, 128])
    c.onesr_d = din("ONESR", [1, 128])
    c.onesc_d = din("ONESC", [128, 1])
    c.rw_d = [din(f"RW{l}", [D, E]) for l in range(L)]
    c.rbb_d = [din(f"RBB{l}", [128, E]) for l in range(L)]
    c.w1h_d = [nc.dram_tensor(f"W1H_{l}", [E, D, D], BF16,
                               kind="ExternalInput").ap() for l in range(L)]
    c.w1l_d = [nc.dram_tensor(f"W1L_{l}", [E, D, D], BF16,
                               kind="ExternalInput").ap() for l in range(L)]
    c.b1_d = [din(f"B1_{l}", [E, 128, KC]) for l in range(L)]
    c.w2h_d = [nc.dram_tensor(f"W2H_{l}", [E, D, D], BF16,
                               kind="ExternalInput").ap() for l in range(L)]
    c.w2l_d = [nc.dram_tensor(f"W2L_{l}", [E, D, D], BF16,
                               kind="ExternalInput").ap() for l in range(L)]
    c.b2_d = [din(f"B2_{l}", [E, D]) for l in range(L)]
    c.wqk_d = [din(f"WQK{a}", [D, 2 * D]) for a in range(L - 1)]
    c.qkb_d = [din(f"QKB{a}", [128, 16]) for a in range(L - 1)]
    c.wv_d = [din(f"WV{a}", [D, D]) for a in range(L - 1)]
    c.vb_d = [din(f"VB{a}", [1, D]) for a in range(L - 1)]
    c.wo_d = [din(f"WO{a}", [D, D]) for a in range(L - 1)]
    c.ob_d = [din(f"OB{a}", [1, D]) for a in range(L - 1)]
    c.y_out = nc.dram_tensor("Y", [TC, D], F32, kind="ExternalOutput").ap()
    c.lg_out = nc.dram_tensor("LG", [L, TC, E], F32,
                              kind="ExternalOutput").ap()

    with ExitStack() as st:
        tc = st.enter_context(tile.TileContext(nc))
        pool = lambda name, bufs, **kw: st.enter_context(
            tc.tile_pool(name=name, bufs=bufs, **kw))
        cpool = pool("consts", 1)
        c.xtp = pool("xtp", 2)
        c.qtp = pool("qtp", 1)
        c.htp = pool("htp", 1)
        c.wexp = pool("wexp", 3)
        c.xnat = pool("xnat", 2)
        c.natp = pool("natp", 1)
        c.epool = pool("epool", 2)
        c.kvp = pool("kvp", 2)
        c.rows = pool("rows", 1)
        c.small = pool("small", 3)
        c.psA = pool("psA", 2, space="PSUM")
        c.pse = pool("pse", 2, space="PSUM")
        c.psav = pool("psav", 1, space="PSUM")
        c.psden = pool("psden", 1, space="PSUM")
        c.dpool = pool("dram", 2, space="DRAM")

        c.lng = cpool.tile([128, D], F32, tag="lng")
        nc.sync.dma_start(c.lng, c.lng_d)
        c.lnb = cpool.tile([128, D], F32, tag="lnb")
        nc.sync.dma_start(c.lnb, c.lnb_d)
        c.eye = cpool.tile([128, 128], F32, tag="eye")
        nc.sync.dma_start(c.eye, c.eye_d)
        c.eyer = cpool.tile([128, 128], F32R, tag="eyer")
        nc.sync.dma_start(c.eyer, c.eye_d.bitcast(F32R))
        c.onesr = cpool.tile([1, 128], F32R, tag="onesr")
        nc.sync.dma_start(c.onesr, c.onesr_d.bitcast(F32R))
        c.onesc = cpool.tile([128, 1], F32R, tag="onesc")
        nc.sync.dma_start(c.onesc, c.onesc_d.bitcast(F32R))
        c.ibt = cpool.tile([128, KC], F32, tag="ibt")
        nc.sync.dma_start(c.ibt, c.ib_d)
        c.epsb = cpool.tile([128, 1], F32, tag="epsb")
        nc.vector.memset(c.epsb, EPS)

        _body(c)

    nc.compile()
    return nc


def _host_prep(params):
    g = {}
    p = params
    asnp = lambda a: np.ascontiguousarray(np.asarray(a, dtype=np.float32))
    g["WI"] = asnp(np.asarray(p["inp_w"]).T)
    g["IB"] = asnp(np.asarray(p["inp_b"]).reshape(KC, 128).T)
    g["LNG"] = asnp(np.tile(np.asarray(p["ln_g"])[None, :], (128, 1)))
    g["LNB"] = asnp(np.tile(np.asarray(p["ln_b"])[None, :], (128, 1)))
    g["EYE"] = np.eye(128, dtype=np.float32)
    g["ONESR"] = np.ones((1, 128), np.float32)
    g["ONESC"] = np.ones((128, 1), np.float32)
    for l in range(L):
        lay = p["layers"][l]
        g[f"RW{l}"] = asnp(lay["router_w"])
        g[f"RBB{l}"] = asnp(np.tile(np.asarray(lay["router_b"])[None, :],
                                    (128, 1)))
        import ml_dtypes
        w1 = np.asarray(lay["w1"], dtype=np.float32)
        w1h = w1.astype(ml_dtypes.bfloat16)
        g[f"W1H_{l}"] = np.ascontiguousarray(w1h)
        g[f"W1L_{l}"] = np.ascontiguousarray(
            (w1 - w1h.astype(np.float32)).astype(ml_dtypes.bfloat16))
        g[f"B1_{l}"] = asnp(np.asarray(lay["b1"]).reshape(E, KC, 128)
                            .transpose(0, 2, 1))
        w2 = np.asarray(lay["w2"], dtype=np.float32)
        w2h = w2.astype(ml_dtypes.bfloat16)
        g[f"W2H_{l}"] = np.ascontiguousarray(w2h)
        g[f"W2L_{l}"] = np.ascontiguousarray(
            (w2 - w2h.astype(np.float32)).astype(ml_dtypes.bfloat16))
        g[f"B2_{l}"] = asnp(lay["b2"])
    for a in range(L - 1):
        att = p["attn"][a]
        qkv_w = np.asarray(att["qkv_w"])
        qkv_b = np.asarray(att["qkv_b"])
        g[f"WQK{a}"] = asnp(qkv_w[:2 * D].T)
        g[f"QKB{a}"] = asnp(qkv_b[:2 * D].reshape(16, 128).T)
        g[f"WV{a}"] = asnp(qkv_w[2 * D:].T)
        g[f"VB{a}"] = asnp(qkv_b[2 * D:][None, :])
        g[f"WO{a}"] = asnp(np.asarray(att["out_w"]).T)
        g[f"OB{a}"] = asnp(np.asarray(att["out_b"])[None, :])
    return g


def kernel(x, params, _trace=False):
    x = np.asarray(x, dtype=np.float32)
    if "nc" not in _CACHE:
        _CACHE["nc"] = build_nc()
    nc = _CACHE["nc"]
    shared = _host_prep(params)
    in_maps = []
    for core in range(NC):
        b, chunk = core // 4, core % 4
        m = dict(shared)
        m["XT0"] = np.ascontiguousarray(x[b, chunk * TC:(chunk + 1) * TC, :].T)
        in_maps.append(m)
    res = bass_utils.run_bass_kernel_spmd(
        nc, in_maps, core_ids=list(range(NC)), trace=_trace)
    x_out = np.zeros((B, S, D), np.float32)
    lg = np.zeros((L, B, S, E), np.float32)
    for core in range(NC):
        b, chunk = core // 4, core % 4
        sl = slice(chunk * TC, (chunk + 1) * TC)
        x_out[b, sl, :] = res.results[core]["Y"]
        lg[:, b, sl, :] = res.results[core]["LG"]
    if _trace:
        _CACHE["last_result"] = res
    return x_out, [lg[i] for i in range(L)]


# revision 22
# speedup vs baseline: 1.2610x; 1.0253x over previous
"""Trainium2 Bass kernel for nn_MoEModel_18476949307967.

4-layer MoE transformer: B=2, S=2048, D=1024, E=8 experts top-2, H=8 heads,
3 interleaved attention layers. 8-way data-parallel over tokens (4096 tokens
-> 512/core; cores 0-3 = batch 0, cores 4-7 = batch 1). Attention K/V is
all-gathered within each 4-core batch group. Dense all-expert FFN per core
with top-2 masked combine (matches the reference exactly).

Precision scheme: the expert FFN runs as exact bf16 hi/lo split matmuls
(W = Wh + Wl and x = xh + xl pre-split; W^T x ~= Wh^T xh + Wh^T xl + Wl^T xh
to ~2^-16), which costs 3 bf16 PE cycles/row instead of fp32's 4 with no
loss of top-2 router-selection fidelity. Attention/projections stay fp32.
Plain f32r (tf32) matmuls would be 4x faster still but their ~2e-4 logit
error flips a couple of near-tied top-2 router selections vs the reference.

Self-contained: hardcodes all shapes; no imports from the problem dir.
"""
from contextlib import ExitStack

import numpy as np

import concourse.bass as bass
import concourse.mybir as mybir
import concourse.tile as tile
from concourse import bacc, bass_utils
from concourse.bass import ts, ds

F32 = mybir.dt.float32
BF16 = mybir.dt.bfloat16
# Matmul operand dtype: float32 is exact (matches the reference's expert
# top-2 selections bit-for-bit in practice) at 4 PE cycles/row; float32r
# (tf32-like) runs 4x faster but its ~2e-4 logit error flips a couple of
# near-tied top-2 router selections vs the fp32 reference.
F32R = mybir.dt.float32
AF = mybir.ActivationFunctionType
ALU = mybir.AluOpType
AX = mybir.AxisListType

B, S, D, E, K, L, H = 2, 2048, 1024, 8, 2, 4, 8
HD = D // H                       # 128
NC = 8                            # cores
TC = (B * S) // NC                # 512 tokens per core
TT = TC // 128                    # 4 token tiles
KC = D // 128                     # 8 contraction subtiles
GROUP = [[0, 1, 2, 3], [4, 5, 6, 7]]
INV_SQRT_HD = 1.0 / float(np.sqrt(HD))
EPS = 1e-5
KV_HALF = D * TC

_CACHE = {}


class _Ctx:
    pass


def _load_wh(c, w2d, col_off, kh):
    """[128, 4, 512] f32r tile <- w2d[kh*512:(kh+1)*512, col_off:col_off+512]."""
    t = c.wexp.tile([128, 4, 512], F32R, tag="wh")
    c.nc.sync.dma_start(
        t, w2d[ds(kh * 512, 512), ds(col_off, 512)]
        .rearrange("(kc p) f -> p kc f", p=128).bitcast(F32R))
    return t


def _load_bf(c, wh_d, wl_d, col_off, kh):
    """One [128, 4, 2, 512] bf16 tile: [:, :, 0, :] = hi, [:, :, 1, :] = lo."""
    t = c.wexp.tile([128, 4, 2, 512], BF16, tag="wh")
    sl = lambda d: d[ds(kh * 512, 512), ds(col_off, 512)].rearrange(
        "(kc p) f -> p kc f", p=128)
    c.nc.sync.dma_start(t[:, :, 0, :], sl(wh_d))
    c.nc.sync.dma_start(t[:, :, 1, :], sl(wl_d))
    return t


def _mm24(nc, ps, tA, tB, lhs_fn, rhs_fn, tail=None):
    """ps = sum over 3 bf16 hi/lo product terms x 8 kc subtiles.
    lhs_fn(t, kc, hl) / rhs_fn(t, kc, hl) select the bf16 operand slices."""
    n = 0
    for lhl, rhl in ((0, 0), (0, 1), (1, 0)):
        for kc in range(KC):
            t = tA if kc < 4 else tB
            n += 1
            nc.tensor.matmul(ps, lhsT=lhs_fn(t, kc, lhl),
                             rhs=rhs_fn(t, kc, rhl),
                             start=(n == 1),
                             stop=(n == 24 and tail is None))
    if tail is not None:
        nc.tensor.matmul(ps, lhsT=tail[0], rhs=tail[1], start=False, stop=True)


def _split_bf(c, src_T, pool, tag):
    """Split a [128, KC, TC] fp32 tile into a [128, KC, 2, TC] bf16 hi/lo."""
    nc = c.nc
    t = pool.tile([128, KC, 2, TC], BF16, tag=tag)
    for k in range(KC):
        nc.vector.tensor_copy(t[:, k, 0, :], src_T[:, k, :])
        nc.vector.tensor_tensor(t[:, k, 1, :], src_T[:, k, :],
                                t[:, k, 0, :], op=ALU.subtract)
    return t


def _mm8(nc, ps, tA, tB, lhs_fn, rhs_fn, tail=None):
    """ps = sum_{kc<8} lhs(t, kc).T @ rhs(kc), lhsT slices from tA/tB halves."""
    for kc in range(KC):
        t = tA if kc < 4 else tB
        last = (kc == KC - 1) and tail is None
        nc.tensor.matmul(ps, lhsT=lhs_fn(t, kc), rhs=rhs_fn(t, kc),
                         start=(kc == 0), stop=last)
    if tail is not None:
        nc.tensor.matmul(ps, lhsT=tail[0], rhs=tail[1], start=False, stop=True)


def _transpose_nat_to_T(c, src_nat, dst_T):
    nc = c.nc
    for m in range(KC):
        for tt in range(TT):
            pt = c.psA.tile([128, 512], F32, tag="psA")
            nc.tensor.transpose(pt[:, :128], src_nat[:, tt, ts(m, 128)], c.eye)
            nc.vector.tensor_copy(dst_T[:, m, ts(tt, 128)], pt[:, :128])


def _input_proj(c):
    nc = c.nc
    x0 = c.xtp.tile([128, KC, TC], F32R, tag="bigT")
    nc.sync.dma_start(
        x0, c.xt0_d.rearrange("(kc p) t -> p kc t", p=128).bitcast(F32R))
    xT = c.xtp.tile([128, KC, TC], F32R, tag="bigT")
    for half in range(2):
        tA = _load_wh(c, c.wi_d, half * 512, 0)
        tB = _load_wh(c, c.wi_d, half * 512, 1)
        for mm in range(4):
            m = half * 4 + mm
            ps = c.psA.tile([128, 512], F32, tag="psA")
            _mm8(nc, ps, tA, tB,
                 lambda t, k, _m=mm: t[:, k % 4, ts(_m, 128)],
                 lambda t, k: x0[:, k, :])
            nc.vector.tensor_scalar(
                xT[:, m, :], ps, c.ibt[:, m:m + 1], None, op0=ALU.add)
    x_n = c.xnat.tile([128, TT, D], F32, tag="xnat")
    for m in range(KC):
        for tt in range(TT):
            pt = c.psA.tile([128, 512], F32, tag="psA")
            nc.tensor.matmul(pt[:, :128], lhsT=xT[:, m, ts(tt, 128)],
                             rhs=c.eyer, start=True, stop=True)
            nc.vector.tensor_copy(x_n[:, tt, ts(m, 128)], pt[:, :128])
    return xT, x_n


def _router(c, l, xT):
    nc = c.nc
    rw = c.small.tile([128, KC, E], F32R, tag="rw")
    nc.sync.dma_start(
        rw, c.rw_d[l].rearrange("(kc p) e -> p kc e", p=128).bitcast(F32R))
    rbb = c.small.tile([128, E], F32, tag="rbb")
    nc.sync.dma_start(rbb, c.rbb_d[l])
    coef = c.small.tile([128, TT, E], F32, tag="coef")
    for tt in range(TT):
        psl = c.psA.tile([128, 512], F32, tag="psA")
        for k in range(KC):
            nc.tensor.matmul(psl[:, :E], lhsT=xT[:, k, ts(tt, 128)],
                             rhs=rw[:, k, :], start=(k == 0),
                             stop=(k == KC - 1))
        lgt = c.small.tile([128, E], F32, tag="lgt")
        nc.vector.tensor_tensor(lgt, psl[:, :E], rbb, op=ALU.add)
        nc.sync.dma_start(c.lg_out[l, ts(tt, 128), :], lgt)
        m8 = c.small.tile([128, 8], F32, tag="m8")
        nc.vector.max(m8, lgt)
        negmax = c.small.tile([128, 1], F32, tag="negmax")
        nc.vector.tensor_scalar_mul(negmax, m8[:, 0:1], -1.0)
        elg = c.small.tile([128, E], F32, tag="elg")
        denl = c.small.tile([128, 1], F32, tag="denl")
        nc.scalar.activation(elg, lgt, AF.Exp, bias=negmax, scale=1.0,
                             accum_out=denl)
        rcl = c.small.tile([128, 1], F32, tag="rcl")
        nc.vector.reciprocal(rcl, denl)
        probs = c.small.tile([128, E], F32, tag="probs")
        nc.vector.tensor_scalar_mul(probs, elg, rcl)
        nc.vector.scalar_tensor_tensor(
            coef[:, tt, :], lgt, m8[:, 1:2], probs,
            op0=ALU.is_ge, op1=ALU.mult)
    return coef


def _moe_experts(c, l, xT, coef):
    nc = c.nc
    y = c.natp.tile([128, TT, D], F32, tag="nat")
    # coefT [E, tok] for the single rank-8 bias matmul sum_e coef_e * b2_e
    coefT = c.rows.tile([8, TT, 128], F32R, tag="coefT")
    for tt in range(TT):
        pt = c.psA.tile([128, 512], F32, tag="psA")
        nc.tensor.transpose(pt[:8, :128], coef[:, tt, :], c.eye)
        nc.vector.tensor_copy(coefT[:, tt, :], pt[:8, :128])
    b2all = c.rows.tile([8, D], F32R, tag="b2all")
    nc.sync.dma_start(b2all, c.b2_d[l][:].bitcast(F32R))
    # split activations into exact bf16 hi/lo: x = xh + xl + O(2^-16)
    xbf = c.xtp.tile([128, KC, 2, TC], BF16, tag="bigT")
    for k in range(KC):
        nc.vector.tensor_copy(xbf[:, k, 0, :], xT[:, k, :])
        nc.vector.tensor_tensor(xbf[:, k, 1, :], xT[:, k, :],
                                xbf[:, k, 0, :], op=ALU.subtract)
    for e in range(E):
        b1t = c.small.tile([128, KC], F32, tag="b1t")
        nc.sync.dma_start(b1t, c.b1_d[l][e])
        hbf = c.htp.tile([128, KC, 2, TC], BF16, tag="hT")
        for half in range(2):
            tA = _load_bf(c, c.w1h_d[l][e], c.w1l_d[l][e], half * 512, 0)
            tB = _load_bf(c, c.w1h_d[l][e], c.w1l_d[l][e], half * 512, 1)
            for mm in range(4):
                m = half * 4 + mm
                ps = c.psA.tile([128, 512], F32, tag="psA")
                _mm24(nc, ps, tA, tB,
                      lambda t, k, hl, _m=mm: t[:, k % 4, hl, ts(_m, 128)],
                      lambda t, k, hl: xbf[:, k, hl, :])
                hrelu = c.kvp.tile([128, 512], F32, tag="hrelu")
                nc.vector.tensor_scalar(
                    hrelu, ps, b1t[:, m:m + 1], 0.0,
                    op0=ALU.add, op1=ALU.max)
                nc.vector.tensor_copy(hbf[:, m, 0, :], hrelu)
                nc.vector.tensor_tensor(hbf[:, m, 1, :], hrelu,
                                        hbf[:, m, 0, :], op=ALU.subtract)
        for half in range(2):
            tA = _load_bf(c, c.w2h_d[l][e], c.w2l_d[l][e], half * 512, 0)
            tB = _load_bf(c, c.w2h_d[l][e], c.w2l_d[l][e], half * 512, 1)
            for tt in range(TT):
                ps = c.psA.tile([128, 512], F32, tag="psA")
                _mm24(nc, ps, tA, tB,
                      lambda t, k, hl, _tt=tt: hbf[:, k, hl, ts(_tt, 128)],
                      lambda t, k, hl: t[:, k % 4, hl, :])
                dst = y[:, tt, ds(half * 512, 512)]
                if e == 0:
                    nc.vector.tensor_scalar_mul(dst, ps, coef[:, tt, 0:1])
                else:
                    nc.vector.scalar_tensor_tensor(
                        dst, ps, coef[:, tt, e:e + 1], dst,
                        op0=ALU.mult, op1=ALU.add)
    # y += coefT.T @ b2 (covers the per-expert b2 bias of every selected
    # expert in one rank-8 matmul per [tok, d-chunk] tile)
    for tt in range(TT):
        for ch in range(2):
            psb = c.psA.tile([128, 512], F32, tag="psA")
            nc.tensor.matmul(psb, lhsT=coefT[:, tt, :],
                             rhs=b2all[:, ds(ch * 512, 512)],
                             start=True, stop=True)
            nc.vector.tensor_tensor(
                y[:, tt, ds(ch * 512, 512)], y[:, tt, ds(ch * 512, 512)],
                psb, op=ALU.add)
    return y


def _attn_qkv(c, a, yT, kvin_k, kvin_v, kvout_k, kvout_v):
    """K projections first so AG(K) overlaps Q/V compute; AG(V) follows the
    V projections and overlaps the first heads' score/exp work."""
    nc = c.nc
    kv_k = kvin_k[:].rearrange("(j t) -> j t", t=TC)
    kv_v = kvin_v[:].rearrange("(t d) -> t d", d=D)
    qkb = c.small.tile([128, 16], F32, tag="qkb")
    nc.sync.dma_start(qkb, c.qkb_d[a])
    vbr = c.rows.tile([1, D], F32R, tag="vbr")
    nc.sync.dma_start(vbr, c.vb_d[a].bitcast(F32R))
    ybf = _split_bf(c, yT, c.xtp, "bigT")
    qT = c.qtp.tile([128, KC, TC], F32R, tag="qT")
    for blk in (2, 3, 0, 1):
        tA = _load_bf(c, c.wqkh_d[a], c.wqkl_d[a], blk * 512, 0)
        tB = _load_bf(c, c.wqkh_d[a], c.wqkl_d[a], blk * 512, 1)
        for sub in range(4):
            jt = blk * 4 + sub
            ps = c.psA.tile([128, 512], F32, tag="psA")
            _mm24(nc, ps, tA, tB,
                  lambda t, k, hl, _s=sub: t[:, k % 4, hl, ts(_s, 128)],
                  lambda t, k, hl: ybf[:, k, hl, :])
            if jt < 8:
                nc.vector.tensor_scalar(
                    qT[:, jt, :], ps, qkb[:, jt:jt + 1], None, op0=ALU.add)
            else:
                kev = c.kvp.tile([128, TC], F32R, tag="kvev")
                nc.vector.tensor_scalar(
                    kev, ps, qkb[:, jt:jt + 1], None, op0=ALU.add)
                nc.sync.dma_start(kv_k[ts(jt - 8, 128), :].bitcast(F32R), kev)
    for half in range(2):
        tA = _load_bf(c, c.wvh_d[a], c.wvl_d[a], half * 512, 0)
        tB = _load_bf(c, c.wvh_d[a], c.wvl_d[a], half * 512, 1)
        for tt in range(TT):
            ps = c.psA.tile([128, 512], F32, tag="psA")
            _mm24(nc, ps, tA, tB,
                  lambda t, k, hl, _tt=tt: ybf[:, k, hl, ts(_tt, 128)],
                  lambda t, k, hl: t[:, k % 4, hl, :],
                  tail=(c.onesr, vbr[:, ds(half * 512, 512)]))
            vev = c.kvp.tile([128, TC], F32R, tag="kvev")
            nc.vector.tensor_copy(vev, ps)
            nc.sync.dma_start(
                kv_v[ts(tt, 128), ds(half * 512, 512)].bitcast(F32R), vev)
    return qT


def _attn_heads(c, qT, kvout_k, kvout_v):
    nc = c.nc
    oT = c.xtp.tile([128, KC, TC], F32R, tag="bigT")
    for h in range(H):
        av = c.psav.tile([128, 512], F32, tag="av")
        den = c.psden.tile([1, 512], F32, tag="den")
        # e-tile sum on DVE so the softmax denominator needs one PE matmul
        eacc = c.kvp.tile([128, 512], F32, tag="eacc")
        nmm = 0
        for r in range(4):
            k_r = kvout_k[r].rearrange("(j t) -> j t", t=TC)
            v_r = kvout_v[r].rearrange(
                "(st p hh hd) -> p st hh hd", p=128, hh=H, hd=HD)
            kblk = c.kvp.tile([128, TC], F32R, tag="kvev")
            nc.sync.dma_start(kblk, k_r[ts(h, 128), :].bitcast(F32R))
            vblk = c.kvp.tile([128, 4, HD], F32R, tag="vblk")
            nc.sync.dma_start(vblk, v_r[:, :, h, :].bitcast(F32R))
            for pair in range(2):
                epst = c.pse.tile([128, 2, 512], F32, tag="eps")
                for sub in range(2):
                    st = pair * 2 + sub
                    nc.tensor.matmul(
                        epst[:, sub, :], lhsT=kblk[:, ts(st, 128)],
                        rhs=qT[:, h, :], start=True, stop=True)
                et = c.epool.tile([128, 2, 512], F32R, tag="et")
                nc.scalar.activation(et, epst, AF.Exp, scale=INV_SQRT_HD)
                if nmm == 0:
                    nc.vector.tensor_tensor(eacc, et[:, 0, :], et[:, 1, :],
                                            op=ALU.add)
                else:
                    nc.vector.tensor_tensor(eacc, eacc, et[:, 0, :],
                                            op=ALU.add)
                    nc.vector.tensor_tensor(eacc, eacc, et[:, 1, :],
                                            op=ALU.add)
                for sub in range(2):
                    st = pair * 2 + sub
                    nmm += 1
                    nc.tensor.matmul(av, lhsT=vblk[:, st, :],
                                     rhs=et[:, sub, :],
                                     start=(nmm == 1), stop=(nmm == 16))
        nc.tensor.matmul(den, lhsT=c.onesc, rhs=eacc.bitcast(F32R),
                         start=True, stop=True)
        rcp = c.rows.tile([1, 512], F32R, tag="rcp")
        with nc.allow_low_precision(reason="f32r is a 4-byte container"):
            nc.vector.reciprocal(rcp, den)
        bc = c.psA.tile([128, 512], F32, tag="psA")
        nc.tensor.matmul(bc, lhsT=c.onesr, rhs=rcp, start=True, stop=True)
        bcs = c.kvp.tile([128, 512], F32, tag="bcs")
        nc.vector.tensor_copy(bcs, bc)
        nc.vector.tensor_tensor(oT[:, h, :], av, bcs, op=ALU.mult)
    return oT


def _attn_out_ln(c, a, oT, x_n):
    nc = c.nc
    obr = c.rows.tile([1, D], F32R, tag="obr")
    nc.sync.dma_start(obr, c.ob_d[a].bitcast(F32R))
    zn = c.natp.tile([128, TT, D], F32, tag="nat")
    obf = _split_bf(c, oT, c.xtp, "bigT")
    for half in range(2):
        tA = _load_bf(c, c.woh_d[a], c.wol_d[a], half * 512, 0)
        tB = _load_bf(c, c.woh_d[a], c.wol_d[a], half * 512, 1)
        for tt in range(TT):
            ps = c.psA.tile([128, 512], F32, tag="psA")
            _mm24(nc, ps, tA, tB,
                  lambda t, k, hl, _tt=tt: obf[:, k, hl, ts(_tt, 128)],
                  lambda t, k, hl: t[:, k % 4, hl, :],
                  tail=(c.onesr, obr[:, ds(half * 512, 512)]))
            nc.vector.tensor_tensor(
                zn[:, tt, ds(half * 512, 512)], ps,
                x_n[:, tt, ds(half * 512, 512)], op=ALU.add)
    xnn = c.xnat.tile([128, TT, D], F32, tag="xnat")
    for tt in range(TT):
        sm = c.small.tile([128, 1], F32, tag="sm")
        nc.vector.reduce_sum(sm, zn[:, tt, :], axis=AX.X)
        negm = c.small.tile([128, 1], F32, tag="negm")
        nc.vector.tensor_scalar_mul(negm, sm, -1.0 / D)
        nc.vector.tensor_scalar(
            zn[:, tt, :], zn[:, tt, :], negm, None, op0=ALU.add)
        nc.vector.tensor_tensor(
            xnn[:, tt, :], zn[:, tt, :], zn[:, tt, :], op=ALU.mult)
        vs = c.small.tile([128, 1], F32, tag="vs")
        nc.vector.reduce_sum(vs, xnn[:, tt, :], axis=AX.X)
        sd = c.small.tile([128, 1], F32, tag="sd")
        nc.scalar.activation(sd, vs, AF.Sqrt, bias=c.epsb, scale=1.0 / D)
        rstd = c.small.tile([128, 1], F32, tag="rstd")
        nc.vector.reciprocal(rstd, sd)
        nc.vector.scalar_tensor_tensor(
            xnn[:, tt, :], zn[:, tt, :], rstd, c.lng,
            op0=ALU.mult, op1=ALU.mult)
        nc.vector.tensor_tensor(xnn[:, tt, :], xnn[:, tt, :], c.lnb,
                                op=ALU.add)
    return xnn


def _body(c):
    nc = c.nc
    xT, x_n = _input_proj(c)
    for l in range(L):
        coef = _router(c, l, xT)
        y = _moe_experts(c, l, xT, coef)
        if l == L - 1:
            for tt in range(TT):
                nc.sync.dma_start(c.y_out[ts(tt, 128), :], y[:, tt, :])
            break
        a = l
        yT = c.xtp.tile([128, KC, TC], F32R, tag="bigT")
        _transpose_nat_to_T(c, y, yT)
        kvin = c.dpool.tile([2 * KV_HALF], F32, tag="kvin")
        kvout = c.dpool.tile([4, 2 * KV_HALF], F32, tag="kvout")
        qT = _attn_qkv(c, a, yT, kvin[0:KV_HALF], kvin[KV_HALF:],
                       None, None)
        nc.gpsimd.collective_compute(
            "AllGather", ALU.bypass,
            ins=[kvin[:].opt()], outs=[kvout[:].opt()],
            replica_groups=GROUP)
        oT = _attn_heads(c, qT,
                         [kvout[r, 0:KV_HALF] for r in range(4)],
                         [kvout[r, KV_HALF:] for r in range(4)])
        x_n = _attn_out_ln(c, a, oT, x_n)
        xT = c.xtp.tile([128, KC, TC], F32R, tag="bigT")
        _transpose_nat_to_T(c, x_n, xT)


def build_nc():
    nc = bacc.Bacc("TRN2", target_bir_lowering=False, debug=False,
                   num_devices=NC)
    c = _Ctx()
    c.nc = nc

    def din(name, shape):
        return nc.dram_tensor(name, shape, F32, kind="ExternalInput").ap()

    c.xt0_d = din("XT0", [D, TC])
    c.wi_d = din("WI", [D, D])
    c.ib_d = din("IB", [128, KC])
    c.lng_d = din("LNG", [128, D])
    c.lnb_d = din("LNB", [128, D])
    c.eye_d = din("EYE", [128, 128])
    c.onesr_d = din("ONESR", [1, 128])
    c.onesc_d = din("ONESC", [128, 1])
    c.rw_d = [din(f"RW{l}", [D, E]) for l in range(L)]
    c.rbb_d = [din(f"RBB{l}", [128, E]) for l in range(L)]
    c.w1h_d = [nc.dram_tensor(f"W1H_{l}", [E, D, D], BF16,
                               kind="ExternalInput").ap() for l in range(L)]
    c.w1l_d = [nc.dram_tensor(f"W1L_{l}", [E, D, D], BF16,
                               kind="ExternalInput").ap() for l in range(L)]
    c.b1_d = [din(f"B1_{l}", [E, 128, KC]) for l in range(L)]
    c.w2h_d = [nc.dram_tensor(f"W2H_{l}", [E, D, D], BF16,
                               kind="ExternalInput").ap() for l in range(L)]
    c.w2l_d = [nc.dram_tensor(f"W2L_{l}", [E, D, D], BF16,
                               kind="ExternalInput").ap() for l in range(L)]
    c.b2_d = [din(f"B2_{l}", [E, D]) for l in range(L)]
    c.wqkh_d = [nc.dram_tensor(f"WQKH{a}", [D, 2 * D], BF16,
                                kind="ExternalInput").ap() for a in range(L - 1)]
    c.wqkl_d = [nc.dram_tensor(f"WQKL{a}", [D, 2 * D], BF16,
                                kind="ExternalInput").ap() for a in range(L - 1)]
    c.qkb_d = [din(f"QKB{a}", [128, 16]) for a in range(L - 1)]
    c.wvh_d = [nc.dram_tensor(f"WVH{a}", [D, D], BF16,
                               kind="ExternalInput").ap() for a in range(L - 1)]
    c.wvl_d = [nc.dram_tensor(f"WVL{a}", [D, D], BF16,
                               kind="ExternalInput").ap() for a in range(L - 1)]
    c.vb_d = [din(f"VB{a}", [1, D]) for a in range(L - 1)]
    c.woh_d = [nc.dram_tensor(f"WOH{a}", [D, D], BF16,
                               kind="ExternalInput").ap() for a in range(L - 1)]
    c.wol_d = [nc.dram_tensor(f"WOL{a}", [D, D], BF16,
                               kind="ExternalInput").ap() for a in range(L - 1)]
    c.ob_d = [din(f"OB{a}", [1, D]) for a in range(L - 1)]
    c.y_out = nc.dram_tensor("Y", [TC, D], F32, kind="ExternalOutput").ap()
    c.lg_out = nc.dram_tensor("LG", [L, TC, E], F32,
                              kind="ExternalOutput").ap()

    with ExitStack() as st:
        tc = st.enter_context(tile.TileContext(nc))
        pool = lambda name, bufs, **kw: st.enter_context(
            tc.tile_pool(name=name, bufs=bufs, **kw))
        cpool = pool("consts", 1)
        c.xtp = pool("xtp", 2)
        c.qtp = pool("qtp", 1)
        c.htp = pool("htp", 1)
        c.wexp = pool("wexp", 3)
        c.xnat = pool("xnat", 2)
        c.natp = pool("natp", 1)
        c.epool = pool("epool", 2)
        c.kvp = pool("kvp", 2)
        c.rows = pool("rows", 1)
        c.small = pool("small", 3)
        c.psA = pool("psA", 2, space="PSUM")
        c.pse = pool("pse", 2, space="PSUM")
        c.psav = pool("psav", 1, space="PSUM")
        c.psden = pool("psden", 1, space="PSUM")
        c.dpool = pool("dram", 2, space="DRAM")

        c.lng = cpool.tile([128, D], F32, tag="lng")
        nc.sync.dma_start(c.lng, c.lng_d)
        c.lnb = cpool.tile([128, D], F32, tag="lnb")
        nc.sync.dma_start(c.lnb, c.lnb_d)
        c.eye = cpool.tile([128, 128], F32, tag="eye")
        nc.sync.dma_start(c.eye, c.eye_d)
        c.eyer = cpool.tile([128, 128], F32R, tag="eyer")
        nc.sync.dma_start(c.eyer, c.eye_d.bitcast(F32R))
        c.onesr = cpool.tile([1, 128], F32R, tag="onesr")
        nc.sync.dma_start(c.onesr, c.onesr_d.bitcast(F32R))
        c.onesc = cpool.tile([128, 1], F32R, tag="onesc")
        nc.sync.dma_start(c.onesc, c.onesc_d.bitcast(F32R))
        c.ibt = cpool.tile([128, KC], F32, tag="ibt")
        nc.sync.dma_start(c.ibt, c.ib_d)
        c.epsb = cpool.tile([128, 1], F32, tag="epsb")
        nc.vector.memset(c.epsb, EPS)

        _body(c)

    nc.compile()
    return nc


def _host_prep(params):
    g = {}
    p = params
    asnp = lambda a: np.ascontiguousarray(np.asarray(a, dtype=np.float32))
    g["WI"] = asnp(np.asarray(p["inp_w"]).T)
    g["IB"] = asnp(np.asarray(p["inp_b"]).reshape(KC, 128).T)
    g["LNG"] = asnp(np.tile(np.asarray(p["ln_g"])[None, :], (128, 1)))
    g["LNB"] = asnp(np.tile(np.asarray(p["ln_b"])[None, :], (128, 1)))
    g["EYE"] = np.eye(128, dtype=np.float32)
    g["ONESR"] = np.ones((1, 128), np.float32)
    g["ONESC"] = np.ones((128, 1), np.float32)
    for l in range(L):
        lay = p["layers"][l]
        g[f"RW{l}"] = asnp(lay["router_w"])
        g[f"RBB{l}"] = asnp(np.tile(np.asarray(lay["router_b"])[None, :],
                                    (128, 1)))
        import ml_dtypes
        w1 = np.asarray(lay["w1"], dtype=np.float32)
        w1h = w1.astype(ml_dtypes.bfloat16)
        g[f"W1H_{l}"] = np.ascontiguousarray(w1h)
        g[f"W1L_{l}"] = np.ascontiguousarray(
            (w1 - w1h.astype(np.float32)).astype(ml_dtypes.bfloat16))
        g[f"B1_{l}"] = asnp(np.asarray(lay["b1"]).reshape(E, KC, 128)
                            .transpose(0, 2, 1))
        w2 = np.asarray(lay["w2"], dtype=np.float32)
        w2h = w2.astype(ml_dtypes.bfloat16)
        g[f"W2H_{l}"] = np.ascontiguousarray(w2h)
        g[f"W2L_{l}"] = np.ascontiguousarray(
            (w2 - w2h.astype(np.float32)).astype(ml_dtypes.bfloat16))
        g[f"B2_{l}"] = asnp(lay["b2"])
    for a in range(L - 1):
        att = p["attn"][a]
        qkv_w = np.asarray(att["qkv_w"])
        qkv_b = np.asarray(att["qkv_b"])
        import ml_dtypes
        def bfsplit(w, hname, lname):
            wh = w.astype(ml_dtypes.bfloat16)
            g[hname] = np.ascontiguousarray(wh)
            g[lname] = np.ascontiguousarray(
                (w - wh.astype(np.float32)).astype(ml_dtypes.bfloat16))
        bfsplit(np.ascontiguousarray(qkv_w[:2 * D].T.astype(np.float32)),
                f"WQKH{a}", f"WQKL{a}")
        g[f"QKB{a}"] = asnp(qkv_b[:2 * D].reshape(16, 128).T)
        bfsplit(np.ascontiguousarray(qkv_w[2 * D:].T.astype(np.float32)),
                f"WVH{a}", f"WVL{a}")
        g[f"VB{a}"] = asnp(qkv_b[2 * D:][None, :])
        bfsplit(np.ascontiguousarray(
            np.asarray(att["out_w"], dtype=np.float32).T),
            f"WOH{a}", f"WOL{a}")
        g[f"OB{a}"] = asnp(np.asarray(att["out_b"])[None, :])
    return g


def kernel(x, params, _trace=False):
    x = np.asarray(x, dtype=np.float32)
    if "nc" not in _CACHE:
        _CACHE["nc"] = build_nc()
    nc = _CACHE["nc"]
    shared = _host_prep(params)
    in_maps = []
    for core in range(NC):
        b, chunk = core // 4, core % 4
        m = dict(shared)
        m["XT0"] = np.ascontiguousarray(x[b, chunk * TC:(chunk + 1) * TC, :].T)
        in_maps.append(m)
    res = bass_utils.run_bass_kernel_spmd(
        nc, in_maps, core_ids=list(range(NC)), trace=_trace)
    x_out = np.zeros((B, S, D), np.float32)
    lg = np.zeros((L, B, S, E), np.float32)
    for core in range(NC):
        b, chunk = core // 4, core % 4
        sl = slice(chunk * TC, (chunk + 1) * TC)
        x_out[b, sl, :] = res.results[core]["Y"]
        lg[:, b, sl, :] = res.results[core]["LG"]
    if _trace:
        _CACHE["last_result"] = res
    return x_out, [lg[i] for i in range(L)]
